# revision 32
# baseline (speedup 1.0000x reference)
"""Bass/Tile TRN2 kernel for BasicAttention (low-precision streams,
PE-based scores, depth-2 prefetched software pipeline).

att = softmax(tanh(hidden @ W_h.T + p_att_feats) @ W_alpha + mask) @ att_feats

Shapes: B=64, N=2048, H=1024, A=512. Data-parallel over batch across 8
NeuronCores (8 batches per core); weights replicated; no collectives.
Measured ~134-150us/core (baseline all-f32 was 259-310us); rel err 7.2e-3.

Design:
  * host casts the streams: att_feats -> bf16, p_att_feats -> fp8e4m3
    (device rel err 7.2e-3 vs the 2e-2 budget; matches the numpy
    ml_dtypes simulation exactly). HBM: 102MB -> ~43MB/core.
  * p_att host-transposed to [b, j, a=128p, n]: A on partitions, so the
    w_h add is a per-partition scalar and add+tanh fuse into ONE ACT op
    (bias=w_hT column). No DVE elementwise add anywhere.
  * scores = alpha.T @ W_alpha on PE (lhsT=W_alphaT column, rhs=alpha
    chunk; j-outer issue so each tanh chunk feeds 4 matmuls at once;
    4 a-chunks accumulate into four [1,512] PSUM bank rows).
  * score rows drain via DVE copies to a [1,2048] row; one tiny
    GPSIMD-queue DMA transposes it to the region-partition layout
    [128,16] (n = p*16+c) needed as att-matmul lhsT.
  * masking is MULTIPLICATIVE after exp (host sends exp(mask); exact
    since exp(s+m)=exp(s)exp(m)): the ACT exp depends only on the
    transpose, and the DVE stt does expt=eraw*expmask with the rowsum
    accumulated in the same op. Softmax denominator via PE ones-matmul,
    reciprocal on DVE, folded into the DVE PSUM drains; SWDGE output.
  * depth-2 pipeline, both streams prefetched on separate HWDGE queues
    (af on Sync, pT on the ACT queue, issued 2 cycles ahead at cycle
    start). Per cycle k: PE [sum(k-2), att(k-2) x32, scores(k) x16],
    ACT [pT(k+2) issues, tanh(k) x4, exp(k-1)] -- every instruction's
    deps are >= 1 cycle old, so no engine FIFO ever blocks on the
    score chain (any such wait entrains tanh and collapses the
    pipeline; this ordering was worth ~40us).

Engine budget/core per ~13-15us cycle: DMA 5.25MiB (the floor at
~340-420GB/s/core), PE ~11us, ACT ~11us, DVE ~5us.

Layouts: region n = p*16 + c (att_feats/masks/scores); a = j*128 + p
(p_att/W_h).
"""

import numpy as np

B, N, H, A = 64, 2048, 1024, 512
NCORES = 8
BLOC = B // NCORES  # batches per core

P = 128
NT = N // P            # 16 n-columns per partition (att layout)
AJ = A // P            # 4 a-chunks (p_att layout)
NN = N // 512          # 4 score chunks of 512
HC = H // P            # 8 contraction chunks for the w_h matmul
AF_SUP = 4             # columns per att_feats supertile (4 DMAs per batch)

# p_att stream dtype: fp8e4m3 (device rel err 7.2e-3). Flip to bf16 if
# inputs change character.
PA_FP8 = True

_NC_CACHE = {}


def _build_nc():
    import concourse.bass as bass
    import concourse.mybir as mybir
    import concourse.tile as tile
    from concourse import bacc

    dt = mybir.dt
    f32, bf16 = dt.float32, dt.bfloat16
    pa_dt = dt.float8e4 if PA_FP8 else bf16
    AF = mybir.ActivationFunctionType
    OP = mybir.AluOpType

    nc = bacc.Bacc("TRN2", target_bir_lowering=False, debug=False,
                   num_devices=NCORES)

    paT = nc.dram_tensor("p_att_T", [BLOC, AJ, P, N], pa_dt,
                         kind="ExternalInput").ap()
    af = nc.dram_tensor("att_feats", [BLOC, N, H], bf16,
                        kind="ExternalInput").ap()
    whTj = nc.dram_tensor("W_hTj", [AJ, P, HC, P], bf16,
                          kind="ExternalInput").ap()
    hsT = nc.dram_tensor("hidden_T", [P, HC, BLOC], bf16,
                         kind="ExternalInput").ap()
    # exp(att_masks): multiplicative masking (exp(s+m) = exp(s)*exp(m),
    # exact; zeros -> 1.0) applied AFTER exp on DVE, so the ACT exp
    # depends only on the transpose DMA -- not on any late DVE stage
    am = nc.dram_tensor("exp_masks", [P, BLOC, NT], bf16,
                        kind="ExternalInput").ap()
    waT = nc.dram_tensor("W_alphaT", [P, AJ], bf16, kind="ExternalInput").ap()
    out = nc.dram_tensor("att_out", [BLOC, H], f32, kind="ExternalOutput").ap()

    with tile.TileContext(nc) as tc:
        with (
            tc.tile_pool(name="consts", bufs=1) as consts,
            tc.tile_pool(name="patt", bufs=12) as patt_pool,
            tc.tile_pool(name="alpha", bufs=5) as alpha_pool,
            tc.tile_pool(name="afp", bufs=14) as af_pool,
            tc.tile_pool(name="small", bufs=3) as small,
            tc.tile_pool(name="rowp", bufs=2) as row_pool,
            tc.tile_pool(name="arow", bufs=2) as arow_pool,
            tc.tile_pool(name="psc", bufs=4, space="PSUM") as psc_pool,
            tc.tile_pool(name="psatt", bufs=2, space="PSUM") as psatt,
            tc.tile_pool(name="psmisc", bufs=2, space="PSUM") as psmisc,
        ):
            af_r = [af[b, :, :].rearrange("(p c) h -> p c h", c=NT)
                    for b in range(BLOC)]

            # ---------------- prologue ----------------
            # Consts ride the ACT HWDGE queue so the Sync queue starts
            # streaming att_feats from the very first instruction.
            whT_sb = []
            hidT_sb = consts.tile([P, HC, BLOC], bf16)
            for j in range(AJ):
                wt = consts.tile([P, HC, P], bf16, name=f"whT{j}")
                nc.scalar.dma_start(out=wt, in_=whTj[j, :, :, :])
                whT_sb.append(wt)
                if j == 0:
                    nc.scalar.dma_start(out=hidT_sb, in_=hsT)
            waT_sb = consts.tile([P, AJ], bf16)
            nc.scalar.dma_start(out=waT_sb, in_=waT)
            masks_all = consts.tile([P, BLOC, NT], bf16)
            nc.scalar.dma_start(out=masks_all, in_=am)

            # pT rides the ACT HWDGE queue (SWDGE transfers measurably
            # degrade aggregate HBM bandwidth: ~320GB/s while pT went
            # through GPSIMD vs ~420GB/s af-only). Issued TWO cycles
            # ahead at the very start of a cycle; with exp demoted to
            # after the NEXT tanh batch and the transpose on GPSIMD,
            # nothing on the ACT queue can block these issues.
            def dma_patt(b):
                tiles = []
                for j in range(AJ):
                    pt = patt_pool.tile([P, N], pa_dt, tag="patt",
                                        name=f"patt{b}_{j}")
                    nc.scalar.dma_start(out=pt, in_=paT[b, j, :, :])
                    tiles.append(pt)
                return tiles

            def dma_af(b):
                tiles = []
                for st in range(NT // AF_SUP):
                    aft = af_pool.tile([P, AF_SUP, H], bf16, tag="af",
                                       name=f"af{b}_{st}")
                    nc.sync.dma_start(
                        out=aft,
                        in_=af_r[b][:, st * AF_SUP:(st + 1) * AF_SUP, :])
                    tiles.append(aft)
                return tiles

            pt_tiles = {0: dma_patt(0), 1: dma_patt(1)}
            af_tiles = {0: dma_af(0)}

            ones_col = consts.tile([P, 1], f32)
            nc.vector.memset(ones_col, 1.0)

            # w_hT[p, j, b] = sum_h W_h[j*128+p, h] * hidden[b, h]:
            # stationary W_h chunks put A on the OUTPUT partitions,
            # giving the transposed per-partition bias directly.
            whbias = consts.tile([P, AJ, BLOC], f32)
            for j in range(AJ):
                wh_ps = psc_pool.tile([P, BLOC], f32, tag="sc",
                                      name=f"whps{j}")
                for hc in range(HC):
                    nc.tensor.matmul(wh_ps, lhsT=whT_sb[j][:, hc, :],
                                     rhs=hidT_sb[:, hc, :],
                                     start=(hc == 0), stop=(hc == HC - 1))
                nc.scalar.copy(whbias[:, j, :], wh_ps)

            # ---------------- per-cycle phases ----------------
            def patt_compute(b):
                # fused add+tanh per a-chunk (data prefetched 2 cycles ago)
                alphas = []
                for j in range(AJ):
                    ab = alpha_pool.tile([P, N], bf16, tag="alpha",
                                         name=f"alpha{b}_{j}")
                    nc.scalar.activation(ab, pt_tiles[b][j], AF.Tanh,
                                         bias=whbias[:, j, b:b + 1])
                    alphas.append(ab)

                # scores on PE, issued j-OUTER so each tanh chunk feeds 4
                # matmuls immediately (the 4 PSUM banks' accumulation
                # groups interleave; each bank still sees start..stop in
                # order). Row nn = scores for n in [512nn, 512nn+512).
                scps = [psc_pool.tile([1, 512], f32, tag="sc",
                                      name=f"sc{b}_{nn}")
                        for nn in range(NN)]
                for j in range(AJ):
                    for nn in range(NN):
                        nc.tensor.matmul(
                            scps[nn], lhsT=waT_sb[:, j:j + 1],
                            rhs=alphas[j][:, nn * 512:(nn + 1) * 512],
                            start=(j == 0), stop=(j == AJ - 1))
                row4 = row_pool.tile([1, N], f32, tag="row", name=f"row{b}")
                for nn in range(NN):
                    nc.vector.tensor_copy(row4[:, nn * 512:(nn + 1) * 512],
                                          scps[nn])

                # tiny transpose DMA [1,2048] -> [128,16] (n = p*16+c) on
                # the GPSIMD queue so the ACT queue never waits on it
                scT = small.tile([P, NT], f32, tag="scT", name=f"scT{b}")
                nc.gpsimd.dma_start(out=scT, in_=row4)
                return scT

            def exp_phase(b, scT):
                # emitted AFTER tanh(b+1) in the ACT FIFO: the exp's only
                # dep (the transpose) is then a cycle old, so ACT never
                # blocks (an ACT stall here entrains tanh and collapses
                # the pipeline). Mask multiply + rowsum follow on DVE.
                eraw = small.tile([P, NT], bf16, tag="eraw", name=f"eraw{b}")
                nc.scalar.activation(eraw, scT, AF.Exp)
                expt = small.tile([P, NT], bf16, tag="expt", name=f"expt{b}")
                rowsum = small.tile([P, 1], f32, tag="rowsum",
                                    name=f"rowsum{b}")
                nc.vector.scalar_tensor_tensor(
                    out=expt, in0=eraw, scalar=1.0, in1=masks_all[:, b, :],
                    op0=OP.mult, op1=OP.mult, accum_out=rowsum)
                return expt, rowsum

            def sum_phase(b, rowsum):
                # cycle b+1: softmax denominator; inv(b) is ready a full
                # cycle before the att drains need it
                sum_ps = psmisc.tile([1, 1], f32, tag="mm", name=f"sum{b}")
                nc.tensor.matmul(sum_ps, lhsT=rowsum, rhs=ones_col,
                                 start=True, stop=True)
                inv = small.tile([1, 1], f32, tag="inv", name=f"inv{b}")
                nc.vector.reciprocal(inv, sum_ps)
                return inv

            def att_mm(b, expt):
                # cycle b+2: every dependency (expt, af tiles) is >=1
                # cycle old, so the PE never waits here
                att_lo = psatt.tile([1, A], f32, tag="att", name=f"attlo{b}")
                att_hi = psatt.tile([1, A], f32, tag="att", name=f"atthi{b}")
                tiles = af_tiles.pop(b)  # loaded two cycles ago
                t = 0
                for st in range(NT // AF_SUP):
                    aft = tiles[st]
                    for c in range(AF_SUP):
                        lhs = expt[:, t:t + 1]
                        nc.tensor.matmul(att_lo, lhsT=lhs,
                                         rhs=aft[:, c, 0:A],
                                         start=(t == 0), stop=(t == NT - 1))
                        nc.tensor.matmul(att_hi, lhsT=lhs,
                                         rhs=aft[:, c, A:H],
                                         start=(t == 0), stop=(t == NT - 1))
                        t += 1
                return att_lo, att_hi

            def att_drain(b, att_lo, att_hi, inv):
                # drain on DVE with 1/sum folded in; emitted AFTER the
                # cycle's score copies + mask so the output path (never
                # urgent) sits last in the DVE FIFO and cannot delay the
                # score->exp chain
                att_row = arow_pool.tile([1, H], f32, tag="attrow",
                                         name=f"attrow{b}")
                nc.vector.tensor_scalar_mul(att_row[:, 0:A], att_lo, inv)
                nc.vector.tensor_scalar_mul(att_row[:, A:H], att_hi, inv)
                nc.gpsimd.dma_start(out=out[b:b + 1, :], in_=att_row)

            # ---------------- software pipeline (depth 2) ----------------
            # cycle k: sum(k-2) | att(k-2) | stream af(k)/pT(k+2) |
            # tanh(k)+scores(k) | exp(k-1).  The score->expt chain of
            # batch k overlaps the att matmuls of batch k-2 on the PE,
            # and every ACT/PE instruction's deps are >= 1 cycle old.
            scT_d, expt_d, rowsum_d, inv_d = {}, {}, {}, {}
            for k in range(BLOC + 2):
                if k + 2 < BLOC:
                    pt_tiles[k + 2] = dma_patt(k + 2)
                ps = None
                if k >= 2:
                    inv_d[k - 2] = sum_phase(k - 2, rowsum_d.pop(k - 2))
                    ps = att_mm(k - 2, expt_d.pop(k - 2))
                if k < BLOC:
                    if k >= 1:
                        af_tiles[k] = dma_af(k)
                    scT_d[k] = patt_compute(k)
                    pt_tiles.pop(k)
                if k >= 1 and k - 1 < BLOC:
                    expt_d[k - 1], rowsum_d[k - 1] = \
                        exp_phase(k - 1, scT_d.pop(k - 1))
                if ps is not None:
                    att_drain(k - 2, *ps, inv_d.pop(k - 2))

    nc.compile()
    return nc


def _get_nc():
    if "nc" not in _NC_CACHE:
        _NC_CACHE["nc"] = _build_nc()
    return _NC_CACHE["nc"]


def kernel(hidden_states, att_feats, p_att_feats, att_masks, W_h, W_alpha):
    import ml_dtypes
    from concourse.bass_utils import run_bass_kernel_spmd

    nc = _get_nc()
    pa_np = ml_dtypes.float8_e4m3fn if PA_FP8 else ml_dtypes.bfloat16
    hidden_states = np.ascontiguousarray(hidden_states, dtype=np.float32)
    att_feats = np.ascontiguousarray(att_feats, dtype=np.float32)
    p_att_feats = np.ascontiguousarray(p_att_feats, dtype=np.float32)
    att_masks = np.ascontiguousarray(att_masks, dtype=np.float32)
    W_h = np.ascontiguousarray(W_h, dtype=np.float32)
    W_alpha = np.asarray(W_alpha, dtype=np.float32)

    # W_hTj[j, p, hc, m] = W_h[j*128+m, hc*128+p]
    whTj = np.ascontiguousarray(
        W_h.reshape(AJ, P, HC, P).transpose(0, 3, 2, 1)
    ).astype(ml_dtypes.bfloat16)
    waT = np.ascontiguousarray(
        W_alpha.reshape(AJ, P).T).astype(ml_dtypes.bfloat16)

    in_maps = []
    for i in range(NCORES):
        s = slice(i * BLOC, (i + 1) * BLOC)
        # [H, BLOC] -> [P, HC, BLOC]
        hT = np.ascontiguousarray(
            hidden_states[s].T.reshape(HC, P, BLOC).transpose(1, 0, 2)
        ).astype(ml_dtypes.bfloat16)
        # exp(mask), [BLOC, N] -> [P, BLOC, NT] with n = p*16 + c
        amr = np.exp(att_masks[s].reshape(BLOC, P, NT).transpose(1, 0, 2)
                     ).astype(ml_dtypes.bfloat16)
        # [BLOC, N, A] -> [BLOC, AJ, P, N] with a = j*128 + p
        paT = p_att_feats[s].transpose(0, 2, 1).reshape(
            BLOC, AJ, P, N).astype(pa_np)
        in_maps.append({
            "p_att_T": paT,
            "att_feats": att_feats[s].astype(ml_dtypes.bfloat16),
            "hidden_T": hT,
            "att_masks": amr,
            "W_hTj": whTj,
            "W_alphaT": waT,
        })

    global _LAST_IN_MAPS
    _LAST_IN_MAPS = in_maps
    res = run_bass_kernel_spmd(nc, in_maps, core_ids=list(range(NCORES)))
    return np.concatenate(
        [res.results[i]["att_out"] for i in range(NCORES)], axis=0
    ).astype(np.float32)


_LAST_IN_MAPS = None


# revision 33
# speedup vs baseline: 1.1165x; 1.1165x over previous
"""Bass/Tile TRN2 kernel for BasicAttention (low-precision streams,
PE-based scores, depth-2 prefetched software pipeline).

att = softmax(tanh(hidden @ W_h.T + p_att_feats) @ W_alpha + mask) @ att_feats

Shapes: B=64, N=2048, H=1024, A=512. Data-parallel over batch across 8
NeuronCores (8 batches per core); weights replicated; no collectives.
Measured ~134-150us/core (baseline all-f32 was 259-310us); rel err 7.2e-3.

Design:
  * host casts the streams: att_feats -> bf16, p_att_feats -> fp8e4m3
    (device rel err 7.2e-3 vs the 2e-2 budget; matches the numpy
    ml_dtypes simulation exactly). HBM: 102MB -> ~43MB/core.
  * p_att host-transposed to [b, j, a=128p, n]: A on partitions, so the
    w_h add is a per-partition scalar and add+tanh fuse into ONE ACT op
    (bias=w_hT column). No DVE elementwise add anywhere.
  * scores = alpha.T @ W_alpha on PE (lhsT=W_alphaT column, rhs=alpha
    chunk; j-outer issue so each tanh chunk feeds 4 matmuls at once;
    4 a-chunks accumulate into four [1,512] PSUM bank rows).
  * score rows drain via DVE copies to a [1,2048] row; one tiny
    GPSIMD-queue DMA transposes it to the region-partition layout
    [128,16] (n = p*16+c) needed as att-matmul lhsT.
  * masking is MULTIPLICATIVE after exp (host sends exp(mask); exact
    since exp(s+m)=exp(s)exp(m)): the ACT exp depends only on the
    transpose, and the DVE stt does expt=eraw*expmask with the rowsum
    accumulated in the same op. Softmax denominator via PE ones-matmul,
    reciprocal on DVE, folded into the DVE PSUM drains; SWDGE output.
  * depth-2 pipeline, both streams prefetched on separate HWDGE queues
    (af on Sync, pT on the ACT queue, issued 2 cycles ahead at cycle
    start). Per cycle k: PE [sum(k-2), att(k-2) x32, scores(k) x16],
    ACT [pT(k+2) issues, tanh(k) x4, exp(k-1)] -- every instruction's
    deps are >= 1 cycle old, so no engine FIFO ever blocks on the
    score chain (any such wait entrains tanh and collapses the
    pipeline; this ordering was worth ~40us).

Engine budget/core per ~13-15us cycle: DMA 5.25MiB (the floor at
~340-420GB/s/core), PE ~11us, ACT ~11us, DVE ~5us.

Layouts: region n = p*16 + c (att_feats/masks/scores); a = j*128 + p
(p_att/W_h).
"""

import numpy as np

B, N, H, A = 64, 2048, 1024, 512
NCORES = 8
BLOC = B // NCORES  # batches per core

P = 128
NT = N // P            # 16 n-columns per partition (att layout)
AJ = A // P            # 4 a-chunks (p_att layout)
NN = N // 512          # 4 score chunks of 512
HC = H // P            # 8 contraction chunks for the w_h matmul
AF_SUP = 4             # columns per att_feats supertile (4 DMAs per batch)

# p_att stream dtype: fp8e4m3 (device rel err 7.2e-3). Flip to bf16 if
# inputs change character.
PA_FP8 = True

_NC_CACHE = {}


def _build_nc():
    import concourse.bass as bass
    import concourse.mybir as mybir
    import concourse.tile as tile
    from concourse import bacc

    dt = mybir.dt
    f32, bf16 = dt.float32, dt.bfloat16
    pa_dt = dt.float8e4 if PA_FP8 else bf16
    AF = mybir.ActivationFunctionType
    OP = mybir.AluOpType

    nc = bacc.Bacc("TRN2", target_bir_lowering=False, debug=False,
                   num_devices=NCORES)

    paT = nc.dram_tensor("p_att_T", [BLOC, AJ, P, N], pa_dt,
                         kind="ExternalInput").ap()
    af = nc.dram_tensor("att_feats", [BLOC, N, H], bf16,
                        kind="ExternalInput").ap()
    whTj = nc.dram_tensor("W_hTj", [AJ, P, HC, P], bf16,
                          kind="ExternalInput").ap()
    hsT = nc.dram_tensor("hidden_T", [P, HC, BLOC], bf16,
                         kind="ExternalInput").ap()
    # exp(att_masks): multiplicative masking (exp(s+m) = exp(s)*exp(m),
    # exact; zeros -> 1.0) applied AFTER exp on DVE, so the ACT exp
    # depends only on the transpose DMA -- not on any late DVE stage
    am = nc.dram_tensor("exp_masks", [P, BLOC, NT], bf16,
                        kind="ExternalInput").ap()
    waT = nc.dram_tensor("W_alphaT", [P, AJ], bf16, kind="ExternalInput").ap()
    out = nc.dram_tensor("att_out", [BLOC, H], f32, kind="ExternalOutput").ap()

    with tile.TileContext(nc) as tc:
        with (
            tc.tile_pool(name="consts", bufs=1) as consts,
            tc.tile_pool(name="patt", bufs=12) as patt_pool,
            tc.tile_pool(name="alpha", bufs=5) as alpha_pool,
            tc.tile_pool(name="afp", bufs=14) as af_pool,
            tc.tile_pool(name="small", bufs=3) as small,
            tc.tile_pool(name="rowp", bufs=2) as row_pool,
            tc.tile_pool(name="arow", bufs=2) as arow_pool,
            tc.tile_pool(name="psc", bufs=4, space="PSUM") as psc_pool,
            tc.tile_pool(name="psatt", bufs=2, space="PSUM") as psatt,
            tc.tile_pool(name="psmisc", bufs=2, space="PSUM") as psmisc,
        ):
            af_r = [af[b, :, :].rearrange("(p c) h -> p c h", c=NT)
                    for b in range(BLOC)]

            # ---------------- prologue ----------------
            # Sync queue: consts then the first att_feats batch (consts
            # on the scalar queue regressed: they delay the pT prefetch
            # behind 1.3MiB and skew the early pipeline).
            whT_sb = []
            hidT_sb = consts.tile([P, HC, BLOC], bf16)
            for j in range(AJ):
                wt = consts.tile([P, HC, P], bf16, name=f"whT{j}")
                nc.sync.dma_start(out=wt, in_=whTj[j, :, :, :])
                whT_sb.append(wt)
                if j == 0:
                    nc.sync.dma_start(out=hidT_sb, in_=hsT)
            waT_sb = consts.tile([P, AJ], bf16)
            nc.sync.dma_start(out=waT_sb, in_=waT)
            masks_all = consts.tile([P, BLOC, NT], bf16)
            nc.sync.dma_start(out=masks_all, in_=am)

            # pT rides the ACT HWDGE queue (SWDGE transfers measurably
            # degrade aggregate HBM bandwidth: ~320GB/s while pT went
            # through GPSIMD vs ~420GB/s af-only). Issued TWO cycles
            # ahead at the very start of a cycle; with exp demoted to
            # after the NEXT tanh batch and the transpose on GPSIMD,
            # nothing on the ACT queue can block these issues.
            def dma_patt(b):
                tiles = []
                for j in range(AJ):
                    pt = patt_pool.tile([P, N], pa_dt, tag="patt",
                                        name=f"patt{b}_{j}")
                    nc.scalar.dma_start(out=pt, in_=paT[b, j, :, :])
                    tiles.append(pt)
                return tiles

            def dma_af(b):
                tiles = []
                for st in range(NT // AF_SUP):
                    aft = af_pool.tile([P, AF_SUP, H], bf16, tag="af",
                                       name=f"af{b}_{st}")
                    nc.sync.dma_start(
                        out=aft,
                        in_=af_r[b][:, st * AF_SUP:(st + 1) * AF_SUP, :])
                    tiles.append(aft)
                return tiles

            pt_tiles = {0: dma_patt(0), 1: dma_patt(1)}
            af_tiles = {0: dma_af(0)}

            ones_col = consts.tile([P, 1], f32)
            nc.vector.memset(ones_col, 1.0)

            # w_hT[p, j, b] = sum_h W_h[j*128+p, h] * hidden[b, h]:
            # stationary W_h chunks put A on the OUTPUT partitions,
            # giving the transposed per-partition bias directly.
            whbias = consts.tile([P, AJ, BLOC], f32)
            for j in range(AJ):
                wh_ps = psc_pool.tile([P, BLOC], f32, tag="sc",
                                      name=f"whps{j}")
                for hc in range(HC):
                    nc.tensor.matmul(wh_ps, lhsT=whT_sb[j][:, hc, :],
                                     rhs=hidT_sb[:, hc, :],
                                     start=(hc == 0), stop=(hc == HC - 1))
                nc.scalar.copy(whbias[:, j, :], wh_ps)

            # ---------------- per-cycle phases ----------------
            def patt_compute(b):
                # fused add+tanh per a-chunk (data prefetched 2 cycles ago)
                alphas = []
                for j in range(AJ):
                    ab = alpha_pool.tile([P, N], bf16, tag="alpha",
                                         name=f"alpha{b}_{j}")
                    nc.scalar.activation(ab, pt_tiles[b][j], AF.Tanh,
                                         bias=whbias[:, j, b:b + 1])
                    alphas.append(ab)

                # scores on PE, issued j-OUTER so each tanh chunk feeds 4
                # matmuls immediately (the 4 PSUM banks' accumulation
                # groups interleave; each bank still sees start..stop in
                # order). Row nn = scores for n in [512nn, 512nn+512).
                scps = [psc_pool.tile([1, 512], f32, tag="sc",
                                      name=f"sc{b}_{nn}")
                        for nn in range(NN)]
                for j in range(AJ):
                    for nn in range(NN):
                        nc.tensor.matmul(
                            scps[nn], lhsT=waT_sb[:, j:j + 1],
                            rhs=alphas[j][:, nn * 512:(nn + 1) * 512],
                            start=(j == 0), stop=(j == AJ - 1))
                row4 = row_pool.tile([1, N], f32, tag="row", name=f"row{b}")
                for nn in range(NN):
                    nc.vector.tensor_copy(row4[:, nn * 512:(nn + 1) * 512],
                                          scps[nn])

                # tiny transpose DMA [1,2048] -> [128,16] (n = p*16+c) on
                # the GPSIMD queue so the ACT queue never waits on it
                scT = small.tile([P, NT], f32, tag="scT", name=f"scT{b}")
                nc.gpsimd.dma_start(out=scT, in_=row4)
                return scT

            def exp_phase(b, scT):
                # emitted AFTER tanh(b+1) in the ACT FIFO: the exp's only
                # dep (the transpose) is then a cycle old, so ACT never
                # blocks (an ACT stall here entrains tanh and collapses
                # the pipeline). Mask multiply + rowsum follow on DVE.
                eraw = small.tile([P, NT], bf16, tag="eraw", name=f"eraw{b}")
                nc.scalar.activation(eraw, scT, AF.Exp)
                expt = small.tile([P, NT], bf16, tag="expt", name=f"expt{b}")
                rowsum = small.tile([P, 1], f32, tag="rowsum",
                                    name=f"rowsum{b}")
                nc.vector.scalar_tensor_tensor(
                    out=expt, in0=eraw, scalar=1.0, in1=masks_all[:, b, :],
                    op0=OP.mult, op1=OP.mult, accum_out=rowsum)
                return expt, rowsum

            def sum_phase(b, rowsum):
                # cycle b+1: softmax denominator; inv(b) is ready a full
                # cycle before the att drains need it
                sum_ps = psmisc.tile([1, 1], f32, tag="mm", name=f"sum{b}")
                nc.tensor.matmul(sum_ps, lhsT=rowsum, rhs=ones_col,
                                 start=True, stop=True)
                inv = small.tile([1, 1], f32, tag="inv", name=f"inv{b}")
                nc.vector.reciprocal(inv, sum_ps)
                return inv

            def att_mm(b, expt):
                # cycle b+2: every dependency (expt, af tiles) is >=1
                # cycle old, so the PE never waits here
                att_lo = psatt.tile([1, A], f32, tag="att", name=f"attlo{b}")
                att_hi = psatt.tile([1, A], f32, tag="att", name=f"atthi{b}")
                tiles = af_tiles.pop(b)  # loaded two cycles ago
                t = 0
                for st in range(NT // AF_SUP):
                    aft = tiles[st]
                    for c in range(AF_SUP):
                        lhs = expt[:, t:t + 1]
                        nc.tensor.matmul(att_lo, lhsT=lhs,
                                         rhs=aft[:, c, 0:A],
                                         start=(t == 0), stop=(t == NT - 1))
                        nc.tensor.matmul(att_hi, lhsT=lhs,
                                         rhs=aft[:, c, A:H],
                                         start=(t == 0), stop=(t == NT - 1))
                        t += 1
                return att_lo, att_hi

            def att_drain(b, att_lo, att_hi, inv):
                # drain on DVE with 1/sum folded in; emitted AFTER the
                # cycle's score copies + mask so the output path (never
                # urgent) sits last in the DVE FIFO and cannot delay the
                # score->exp chain
                att_row = arow_pool.tile([1, H], f32, tag="attrow",
                                         name=f"attrow{b}")
                nc.vector.tensor_scalar_mul(att_row[:, 0:A], att_lo, inv)
                nc.vector.tensor_scalar_mul(att_row[:, A:H], att_hi, inv)
                nc.gpsimd.dma_start(out=out[b:b + 1, :], in_=att_row)

            # ---------------- software pipeline (depth 2) ----------------
            # cycle k: sum(k-2) | att(k-2) | stream af(k)/pT(k+2) |
            # tanh(k)+scores(k) | exp(k-1).  The score->expt chain of
            # batch k overlaps the att matmuls of batch k-2 on the PE,
            # and every ACT/PE instruction's deps are >= 1 cycle old.
            scT_d, expt_d, rowsum_d, inv_d = {}, {}, {}, {}
            for k in range(BLOC + 2):
                if k + 2 < BLOC:
                    pt_tiles[k + 2] = dma_patt(k + 2)
                ps = None
                if k >= 2:
                    inv_d[k - 2] = sum_phase(k - 2, rowsum_d.pop(k - 2))
                    ps = att_mm(k - 2, expt_d.pop(k - 2))
                if k < BLOC:
                    if k >= 1:
                        af_tiles[k] = dma_af(k)
                    scT_d[k] = patt_compute(k)
                    pt_tiles.pop(k)
                if k >= 1 and k - 1 < BLOC:
                    expt_d[k - 1], rowsum_d[k - 1] = \
                        exp_phase(k - 1, scT_d.pop(k - 1))
                if ps is not None:
                    att_drain(k - 2, *ps, inv_d.pop(k - 2))

    nc.compile()
    return nc


def _get_nc():
    if "nc" not in _NC_CACHE:
        _NC_CACHE["nc"] = _build_nc()
    return _NC_CACHE["nc"]


def kernel(hidden_states, att_feats, p_att_feats, att_masks, W_h, W_alpha):
    import ml_dtypes
    from concourse.bass_utils import run_bass_kernel_spmd

    nc = _get_nc()
    pa_np = ml_dtypes.float8_e4m3fn if PA_FP8 else ml_dtypes.bfloat16
    hidden_states = np.ascontiguousarray(hidden_states, dtype=np.float32)
    att_feats = np.ascontiguousarray(att_feats, dtype=np.float32)
    p_att_feats = np.ascontiguousarray(p_att_feats, dtype=np.float32)
    att_masks = np.ascontiguousarray(att_masks, dtype=np.float32)
    W_h = np.ascontiguousarray(W_h, dtype=np.float32)
    W_alpha = np.asarray(W_alpha, dtype=np.float32)

    # W_hTj[j, p, hc, m] = W_h[j*128+m, hc*128+p]
    whTj = np.ascontiguousarray(
        W_h.reshape(AJ, P, HC, P).transpose(0, 3, 2, 1)
    ).astype(ml_dtypes.bfloat16)
    waT = np.ascontiguousarray(
        W_alpha.reshape(AJ, P).T).astype(ml_dtypes.bfloat16)

    in_maps = []
    for i in range(NCORES):
        s = slice(i * BLOC, (i + 1) * BLOC)
        # [H, BLOC] -> [P, HC, BLOC]
        hT = np.ascontiguousarray(
            hidden_states[s].T.reshape(HC, P, BLOC).transpose(1, 0, 2)
        ).astype(ml_dtypes.bfloat16)
        # exp(mask), [BLOC, N] -> [P, BLOC, NT] with n = p*16 + c
        amr = np.exp(att_masks[s].reshape(BLOC, P, NT).transpose(1, 0, 2)
                     ).astype(ml_dtypes.bfloat16)
        # [BLOC, N, A] -> [BLOC, AJ, P, N] with a = j*128 + p
        paT = p_att_feats[s].transpose(0, 2, 1).reshape(
            BLOC, AJ, P, N).astype(pa_np)
        in_maps.append({
            "p_att_T": paT,
            "att_feats": att_feats[s].astype(ml_dtypes.bfloat16),
            "hidden_T": hT,
            "att_masks": amr,
            "W_hTj": whTj,
            "W_alphaT": waT,
        })

    global _LAST_IN_MAPS
    _LAST_IN_MAPS = in_maps
    res = run_bass_kernel_spmd(nc, in_maps, core_ids=list(range(NCORES)))
    return np.concatenate(
        [res.results[i]["att_out"] for i in range(NCORES)], axis=0
    ).astype(np.float32)


_LAST_IN_MAPS = None


# revision 37
# speedup vs baseline: 1.1551x; 1.0346x over previous
"""Bass/Tile TRN2 kernel for BasicAttention (low-precision streams,
PE-based scores, depth-2 prefetched software pipeline).

att = softmax(tanh(hidden @ W_h.T + p_att_feats) @ W_alpha + mask) @ att_feats

Shapes: B=64, N=2048, H=1024, A=512. Data-parallel over batch across 8
NeuronCores (8 batches per core); weights replicated; no collectives.
Measured ~134-150us/core (baseline all-f32 was 259-310us); rel err 7.2e-3.

Design:
  * host casts the streams: att_feats -> bf16, p_att_feats -> fp8e4m3
    (device rel err 7.2e-3 vs the 2e-2 budget; matches the numpy
    ml_dtypes simulation exactly). HBM: 102MB -> ~43MB/core.
  * p_att host-transposed to [b, j, a=128p, n]: A on partitions, so the
    w_h add is a per-partition scalar and add+tanh fuse into ONE ACT op
    (bias=w_hT column). No DVE elementwise add anywhere.
  * scores = alpha.T @ W_alpha on PE (lhsT=W_alphaT column, rhs=alpha
    chunk; j-outer issue so each tanh chunk feeds 4 matmuls at once;
    4 a-chunks accumulate into four [1,512] PSUM bank rows).
  * score rows drain via DVE copies to a [1,2048] row; one tiny
    GPSIMD-queue DMA transposes it to the region-partition layout
    [128,16] (n = p*16+c) needed as att-matmul lhsT.
  * masking is MULTIPLICATIVE after exp (host sends exp(mask); exact
    since exp(s+m)=exp(s)exp(m)): the ACT exp depends only on the
    transpose, and the DVE stt does expt=eraw*expmask with the rowsum
    accumulated in the same op. Softmax denominator via PE ones-matmul,
    reciprocal on DVE, folded into the DVE PSUM drains; SWDGE output.
  * depth-2 pipeline, both streams prefetched on separate HWDGE queues
    (af on Sync, pT on the ACT queue, issued 2 cycles ahead at cycle
    start). Per cycle k: PE [sum(k-2), att(k-2) x32, scores(k) x16],
    ACT [pT(k+2) issues, tanh(k) x4, exp(k-1)] -- every instruction's
    deps are >= 1 cycle old, so no engine FIFO ever blocks on the
    score chain (any such wait entrains tanh and collapses the
    pipeline; this ordering was worth ~40us).

Engine budget/core per ~13-15us cycle: DMA 5.25MiB (the floor at
~340-420GB/s/core), PE ~11us, ACT ~11us, DVE ~5us.

Layouts: region n = p*16 + c (att_feats/masks/scores); a = j*128 + p
(p_att/W_h).
"""

import numpy as np

B, N, H, A = 64, 2048, 1024, 512
NCORES = 8
BLOC = B // NCORES  # batches per core

P = 128
NT = N // P            # 16 n-columns per partition (att layout)
AJ = A // P            # 4 a-chunks (p_att layout)
NN = N // 512          # 4 score chunks of 512
HC = H // P            # 8 contraction chunks for the w_h matmul
AF_SUP = 4             # columns per att_feats supertile (4 DMAs per batch)

# p_att stream dtype: fp8e4m3 (device rel err 7.2e-3). Flip to bf16 if
# inputs change character.
PA_FP8 = True

_NC_CACHE = {}


def _build_nc():
    import concourse.bass as bass
    import concourse.mybir as mybir
    import concourse.tile as tile
    from concourse import bacc

    dt = mybir.dt
    f32, bf16 = dt.float32, dt.bfloat16
    pa_dt = dt.float8e4 if PA_FP8 else bf16
    AF = mybir.ActivationFunctionType
    OP = mybir.AluOpType

    nc = bacc.Bacc("TRN2", target_bir_lowering=False, debug=False,
                   num_devices=NCORES)

    paT = nc.dram_tensor("p_att_T", [BLOC, AJ, P, N], pa_dt,
                         kind="ExternalInput").ap()
    af = nc.dram_tensor("att_feats", [BLOC, N, H], bf16,
                        kind="ExternalInput").ap()
    whTj = nc.dram_tensor("W_hTj", [AJ, P, HC, P], bf16,
                          kind="ExternalInput").ap()
    hsT = nc.dram_tensor("hidden_T", [P, HC, BLOC], bf16,
                         kind="ExternalInput").ap()
    # exp(att_masks): multiplicative masking (exp(s+m) = exp(s)*exp(m),
    # exact; zeros -> 1.0) applied AFTER exp on DVE, so the ACT exp
    # depends only on the transpose DMA -- not on any late DVE stage
    am = nc.dram_tensor("exp_masks", [P, BLOC, NT], bf16,
                        kind="ExternalInput").ap()
    waT = nc.dram_tensor("W_alphaT", [P, AJ], bf16, kind="ExternalInput").ap()
    out = nc.dram_tensor("att_out", [BLOC, H], f32, kind="ExternalOutput").ap()

    with tile.TileContext(nc) as tc:
        with (
            tc.tile_pool(name="consts", bufs=1) as consts,
            tc.tile_pool(name="patt", bufs=12) as patt_pool,
            tc.tile_pool(name="alpha", bufs=5) as alpha_pool,
            tc.tile_pool(name="afp", bufs=14) as af_pool,
            tc.tile_pool(name="small", bufs=3) as small,
            tc.tile_pool(name="rowp", bufs=2) as row_pool,
            tc.tile_pool(name="arow", bufs=2) as arow_pool,
            tc.tile_pool(name="psc", bufs=4, space="PSUM") as psc_pool,
            tc.tile_pool(name="psatt", bufs=2, space="PSUM") as psatt,
            tc.tile_pool(name="psmisc", bufs=2, space="PSUM") as psmisc,
        ):
            af_r = [af[b, :, :].rearrange("(p c) h -> p c h", c=NT)
                    for b in range(BLOC)]

            # ---------------- prologue ----------------
            # Sync queue: consts then the first att_feats batch (consts
            # on the scalar queue regressed: they delay the pT prefetch
            # behind 1.3MiB and skew the early pipeline).
            whT_sb = []
            hidT_sb = consts.tile([P, HC, BLOC], bf16)
            for j in range(AJ):
                wt = consts.tile([P, HC, P], bf16, name=f"whT{j}")
                nc.sync.dma_start(out=wt, in_=whTj[j, :, :, :])
                whT_sb.append(wt)
                if j == 0:
                    nc.sync.dma_start(out=hidT_sb, in_=hsT)
            waT_sb = consts.tile([P, AJ], bf16)
            nc.sync.dma_start(out=waT_sb, in_=waT)
            masks_all = consts.tile([P, BLOC, NT], bf16)
            nc.sync.dma_start(out=masks_all, in_=am)

            # pT rides the ACT HWDGE queue (SWDGE transfers measurably
            # degrade aggregate HBM bandwidth: ~320GB/s while pT went
            # through GPSIMD vs ~420GB/s af-only). Issued TWO cycles
            # ahead at the very start of a cycle; with exp demoted to
            # after the NEXT tanh batch and the transpose on GPSIMD,
            # nothing on the ACT queue can block these issues.
            def dma_patt(b):
                tiles = []
                for j in range(AJ):
                    pt = patt_pool.tile([P, N], pa_dt, tag="patt",
                                        name=f"patt{b}_{j}")
                    nc.scalar.dma_start(out=pt, in_=paT[b, j, :, :])
                    tiles.append(pt)
                return tiles

            def dma_af(b):
                tiles = []
                for st in range(NT // AF_SUP):
                    aft = af_pool.tile([P, AF_SUP, H], bf16, tag="af",
                                       name=f"af{b}_{st}")
                    nc.sync.dma_start(
                        out=aft,
                        in_=af_r[b][:, st * AF_SUP:(st + 1) * AF_SUP, :])
                    tiles.append(aft)
                return tiles

            pt_tiles = {0: dma_patt(0), 1: dma_patt(1)}
            af_tiles = {0: dma_af(0)}

            ones_col = consts.tile([P, 1], f32)
            nc.vector.memset(ones_col, 1.0)

            # w_hT[p, j, b] = sum_h W_h[j*128+p, h] * hidden[b, h]:
            # stationary W_h chunks put A on the OUTPUT partitions,
            # giving the transposed per-partition bias directly.
            whbias = consts.tile([P, AJ, BLOC], f32)
            for j in range(AJ):
                wh_ps = psc_pool.tile([P, BLOC], f32, tag="sc",
                                      name=f"whps{j}")
                for hc in range(HC):
                    nc.tensor.matmul(wh_ps, lhsT=whT_sb[j][:, hc, :],
                                     rhs=hidT_sb[:, hc, :],
                                     start=(hc == 0), stop=(hc == HC - 1))
                nc.scalar.copy(whbias[:, j, :], wh_ps)

            # ---------------- per-cycle phases ----------------
            def patt_compute(b):
                # fused add+tanh per a-chunk (data prefetched 2 cycles ago)
                alphas = []
                for j in range(AJ):
                    ab = alpha_pool.tile([P, N], bf16, tag="alpha",
                                         name=f"alpha{b}_{j}")
                    nc.scalar.activation(ab, pt_tiles[b][j], AF.Tanh,
                                         bias=whbias[:, j, b:b + 1])
                    alphas.append(ab)

                # scores on PE, issued j-OUTER so each tanh chunk feeds 4
                # matmuls immediately (the 4 PSUM banks' accumulation
                # groups interleave; each bank still sees start..stop in
                # order). Row nn = scores for n in [512nn, 512nn+512).
                scps = [psc_pool.tile([1, 512], f32, tag="sc",
                                      name=f"sc{b}_{nn}")
                        for nn in range(NN)]
                for j in range(AJ):
                    for nn in range(NN):
                        nc.tensor.matmul(
                            scps[nn], lhsT=waT_sb[:, j:j + 1],
                            rhs=alphas[j][:, nn * 512:(nn + 1) * 512],
                            start=(j == 0), stop=(j == AJ - 1))
                row4 = row_pool.tile([1, N], f32, tag="row", name=f"row{b}")
                for nn in range(NN):
                    nc.vector.tensor_copy(row4[:, nn * 512:(nn + 1) * 512],
                                          scps[nn])

                # tiny transpose DMA [1,2048] -> [128,16] (n = p*16+c) on
                # the GPSIMD queue so the ACT queue never waits on it
                scT = small.tile([P, NT], f32, tag="scT", name=f"scT{b}")
                nc.gpsimd.dma_start(out=scT, in_=row4)
                return scT

            def exp_phase(b, scT):
                # emitted AFTER tanh(b+1) in the ACT FIFO: the exp's only
                # dep (the transpose) is then a cycle old, so ACT never
                # blocks (an ACT stall here entrains tanh and collapses
                # the pipeline). Mask multiply + rowsum follow on DVE.
                # NOTE: the DVE stt cannot be emitted earlier than the
                # exp (tile deps follow program order; trying it read
                # uninitialized eraw -> nan).
                eraw = small.tile([P, NT], bf16, tag="eraw", name=f"eraw{b}")
                nc.scalar.activation(eraw, scT, AF.Exp)
                expt = small.tile([P, NT], bf16, tag="expt", name=f"expt{b}")
                rowsum = small.tile([P, 1], f32, tag="rowsum",
                                    name=f"rowsum{b}")
                nc.vector.scalar_tensor_tensor(
                    out=expt, in0=eraw, scalar=1.0, in1=masks_all[:, b, :],
                    op0=OP.mult, op1=OP.mult, accum_out=rowsum)
                return expt, rowsum

            def sum_phase(b, rowsum):
                # cycle b+1: softmax denominator; inv(b) is ready a full
                # cycle before the att drains need it
                sum_ps = psmisc.tile([1, 1], f32, tag="mm", name=f"sum{b}")
                nc.tensor.matmul(sum_ps, lhsT=rowsum, rhs=ones_col,
                                 start=True, stop=True)
                inv = small.tile([1, 1], f32, tag="inv", name=f"inv{b}")
                nc.vector.reciprocal(inv, sum_ps)
                return inv

            def att_mm(b, expt):
                # cycle b+2: every dependency (expt, af tiles) is >=1
                # cycle old, so the PE never waits here
                att_lo = psatt.tile([1, A], f32, tag="att", name=f"attlo{b}")
                att_hi = psatt.tile([1, A], f32, tag="att", name=f"atthi{b}")
                tiles = af_tiles.pop(b)  # loaded two cycles ago
                t = 0
                for st in range(NT // AF_SUP):
                    aft = tiles[st]
                    for c in range(AF_SUP):
                        lhs = expt[:, t:t + 1]
                        nc.tensor.matmul(att_lo, lhsT=lhs,
                                         rhs=aft[:, c, 0:A],
                                         start=(t == 0), stop=(t == NT - 1))
                        nc.tensor.matmul(att_hi, lhsT=lhs,
                                         rhs=aft[:, c, A:H],
                                         start=(t == 0), stop=(t == NT - 1))
                        t += 1
                return att_lo, att_hi

            def att_drain(b, att_lo, att_hi, inv):
                # drain on DVE with 1/sum folded in; emitted AFTER the
                # cycle's score copies + mask so the output path (never
                # urgent) sits last in the DVE FIFO and cannot delay the
                # score->exp chain
                att_row = arow_pool.tile([1, H], f32, tag="attrow",
                                         name=f"attrow{b}")
                nc.vector.tensor_scalar_mul(att_row[:, 0:A], att_lo, inv)
                nc.vector.tensor_scalar_mul(att_row[:, A:H], att_hi, inv)
                nc.gpsimd.dma_start(out=out[b:b + 1, :], in_=att_row)

            # ---------------- software pipeline (depth 2) ----------------
            # cycle k: sum(k-2) | att(k-2) | stream af(k)/pT(k+2) |
            # tanh(k)+scores(k) | exp(k-1).  The score->expt chain of
            # batch k overlaps the att matmuls of batch k-2 on the PE,
            # and every ACT/PE instruction's deps are >= 1 cycle old.
            scT_d, expt_d, rowsum_d, inv_d = {}, {}, {}, {}
            for k in range(BLOC + 2):
                if k + 2 < BLOC:
                    pt_tiles[k + 2] = dma_patt(k + 2)
                ps = None
                if k >= 2:
                    inv_d[k - 2] = sum_phase(k - 2, rowsum_d.pop(k - 2))
                    ps = att_mm(k - 2, expt_d.pop(k - 2))
                if k < BLOC:
                    if k >= 1:
                        af_tiles[k] = dma_af(k)
                    scT_d[k] = patt_compute(k)
                    pt_tiles.pop(k)
                if k >= 1 and k - 1 < BLOC:
                    expt_d[k - 1], rowsum_d[k - 1] = \
                        exp_phase(k - 1, scT_d.pop(k - 1))
                if ps is not None:
                    att_drain(k - 2, *ps, inv_d.pop(k - 2))

    nc.compile()
    return nc


def _get_nc():
    if "nc" not in _NC_CACHE:
        _NC_CACHE["nc"] = _build_nc()
    return _NC_CACHE["nc"]


def kernel(hidden_states, att_feats, p_att_feats, att_masks, W_h, W_alpha):
    import ml_dtypes
    from concourse.bass_utils import run_bass_kernel_spmd

    nc = _get_nc()
    pa_np = ml_dtypes.float8_e4m3fn if PA_FP8 else ml_dtypes.bfloat16
    hidden_states = np.ascontiguousarray(hidden_states, dtype=np.float32)
    att_feats = np.ascontiguousarray(att_feats, dtype=np.float32)
    p_att_feats = np.ascontiguousarray(p_att_feats, dtype=np.float32)
    att_masks = np.ascontiguousarray(att_masks, dtype=np.float32)
    W_h = np.ascontiguousarray(W_h, dtype=np.float32)
    W_alpha = np.asarray(W_alpha, dtype=np.float32)

    # W_hTj[j, p, hc, m] = W_h[j*128+m, hc*128+p]
    whTj = np.ascontiguousarray(
        W_h.reshape(AJ, P, HC, P).transpose(0, 3, 2, 1)
    ).astype(ml_dtypes.bfloat16)
    waT = np.ascontiguousarray(
        W_alpha.reshape(AJ, P).T).astype(ml_dtypes.bfloat16)

    in_maps = []
    for i in range(NCORES):
        s = slice(i * BLOC, (i + 1) * BLOC)
        # [H, BLOC] -> [P, HC, BLOC]
        hT = np.ascontiguousarray(
            hidden_states[s].T.reshape(HC, P, BLOC).transpose(1, 0, 2)
        ).astype(ml_dtypes.bfloat16)
        # exp(mask), [BLOC, N] -> [P, BLOC, NT] with n = p*16 + c
        amr = np.exp(att_masks[s].reshape(BLOC, P, NT).transpose(1, 0, 2)
                     ).astype(ml_dtypes.bfloat16)
        # [BLOC, N, A] -> [BLOC, AJ, P, N] with a = j*128 + p
        paT = p_att_feats[s].transpose(0, 2, 1).reshape(
            BLOC, AJ, P, N).astype(pa_np)
        in_maps.append({
            "p_att_T": paT,
            "att_feats": att_feats[s].astype(ml_dtypes.bfloat16),
            "hidden_T": hT,
            "att_masks": amr,
            "W_hTj": whTj,
            "W_alphaT": waT,
        })

    global _LAST_IN_MAPS
    _LAST_IN_MAPS = in_maps
    res = run_bass_kernel_spmd(nc, in_maps, core_ids=list(range(NCORES)))
    return np.concatenate(
        [res.results[i]["att_out"] for i in range(NCORES)], axis=0
    ).astype(np.float32)


_LAST_IN_MAPS = None


# revision 38
# speedup vs baseline: 1.1910x; 1.0311x over previous
"""Bass/Tile TRN2 kernel for BasicAttention (low-precision streams,
PE-based scores, depth-2 prefetched software pipeline).

att = softmax(tanh(hidden @ W_h.T + p_att_feats) @ W_alpha + mask) @ att_feats

Shapes: B=64, N=2048, H=1024, A=512. Data-parallel over batch across 8
NeuronCores (8 batches per core); weights replicated; no collectives.
Measured ~134-150us/core (baseline all-f32 was 259-310us); rel err 7.2e-3.

Design:
  * host casts the streams: att_feats AND p_att_feats -> fp8e4m3
    (device rel err ~1.83e-2 vs the 2e-2 budget; deterministic and
    matching the numpy ml_dtypes simulation within 0.2%). The att
    matmuls run mixed bf16(lhsT) x fp8(rhs). HBM: 102MB -> ~26MB/core;
    the PE (~95us) becomes the binding budget instead of DMA.
  * p_att host-transposed to [b, j, a=128p, n]: A on partitions, so the
    w_h add is a per-partition scalar and add+tanh fuse into ONE ACT op
    (bias=w_hT column). No DVE elementwise add anywhere.
  * scores = alpha.T @ W_alpha on PE (lhsT=W_alphaT column, rhs=alpha
    chunk; j-outer issue so each tanh chunk feeds 4 matmuls at once;
    4 a-chunks accumulate into four [1,512] PSUM bank rows).
  * score rows drain via DVE copies to a [1,2048] row; one tiny
    GPSIMD-queue DMA transposes it to the region-partition layout
    [128,16] (n = p*16+c) needed as att-matmul lhsT.
  * masking is MULTIPLICATIVE after exp (host sends exp(mask); exact
    since exp(s+m)=exp(s)exp(m)): the ACT exp depends only on the
    transpose, and the DVE stt does expt=eraw*expmask with the rowsum
    accumulated in the same op. Softmax denominator via PE ones-matmul,
    reciprocal on DVE, folded into the DVE PSUM drains; SWDGE output.
  * depth-2 pipeline, both streams prefetched on separate HWDGE queues
    (af on Sync, pT on the ACT queue, issued 2 cycles ahead at cycle
    start). Per cycle k: PE [sum(k-2), att(k-2) x32, scores(k) x16],
    ACT [pT(k+2) issues, tanh(k) x4, exp(k-1)] -- every instruction's
    deps are >= 1 cycle old, so no engine FIFO ever blocks on the
    score chain (any such wait entrains tanh and collapses the
    pipeline; this ordering was worth ~40us).

Engine budget/core per ~13-15us cycle: DMA 5.25MiB (the floor at
~340-420GB/s/core), PE ~11us, ACT ~11us, DVE ~5us.

Layouts: region n = p*16 + c (att_feats/masks/scores); a = j*128 + p
(p_att/W_h).
"""

import numpy as np

B, N, H, A = 64, 2048, 1024, 512
NCORES = 8
BLOC = B // NCORES  # batches per core

P = 128
NT = N // P            # 16 n-columns per partition (att layout)
AJ = A // P            # 4 a-chunks (p_att layout)
NN = N // 512          # 4 score chunks of 512
HC = H // P            # 8 contraction chunks for the w_h matmul
AF_SUP = 4             # columns per att_feats supertile (4 DMAs per batch)

# p_att stream dtype: fp8e4m3 (device rel err 7.2e-3). Flip to bf16 if
# inputs change character.
PA_FP8 = True

_NC_CACHE = {}


def _build_nc():
    import concourse.bass as bass
    import concourse.mybir as mybir
    import concourse.tile as tile
    from concourse import bacc

    dt = mybir.dt
    f32, bf16 = dt.float32, dt.bfloat16
    pa_dt = dt.float8e4 if PA_FP8 else bf16
    AF = mybir.ActivationFunctionType
    OP = mybir.AluOpType

    nc = bacc.Bacc("TRN2", target_bir_lowering=False, debug=False,
                   num_devices=NCORES)

    paT = nc.dram_tensor("p_att_T", [BLOC, AJ, P, N], pa_dt,
                         kind="ExternalInput").ap()
    af = nc.dram_tensor("att_feats", [BLOC, N, H], dt.float8e4,
                        kind="ExternalInput").ap()
    whTj = nc.dram_tensor("W_hTj", [AJ, P, HC, P], bf16,
                          kind="ExternalInput").ap()
    hsT = nc.dram_tensor("hidden_T", [P, HC, BLOC], bf16,
                         kind="ExternalInput").ap()
    # exp(att_masks): multiplicative masking (exp(s+m) = exp(s)*exp(m),
    # exact; zeros -> 1.0) applied AFTER exp on DVE, so the ACT exp
    # depends only on the transpose DMA -- not on any late DVE stage
    am = nc.dram_tensor("exp_masks", [P, BLOC, NT], bf16,
                        kind="ExternalInput").ap()
    waT = nc.dram_tensor("W_alphaT", [P, AJ], bf16, kind="ExternalInput").ap()
    out = nc.dram_tensor("att_out", [BLOC, H], f32, kind="ExternalOutput").ap()

    with tile.TileContext(nc) as tc:
        with (
            tc.tile_pool(name="consts", bufs=1) as consts,
            tc.tile_pool(name="patt", bufs=12) as patt_pool,
            tc.tile_pool(name="alpha", bufs=5) as alpha_pool,
            tc.tile_pool(name="afp", bufs=14) as af_pool,
            tc.tile_pool(name="small", bufs=3) as small,
            tc.tile_pool(name="rowp", bufs=2) as row_pool,
            tc.tile_pool(name="arow", bufs=2) as arow_pool,
            tc.tile_pool(name="psc", bufs=4, space="PSUM") as psc_pool,
            tc.tile_pool(name="psatt", bufs=2, space="PSUM") as psatt,
            tc.tile_pool(name="psmisc", bufs=2, space="PSUM") as psmisc,
        ):
            af_r = [af[b, :, :].rearrange("(p c) h -> p c h", c=NT)
                    for b in range(BLOC)]

            # ---------------- prologue ----------------
            # Sync queue: consts then the first att_feats batch (consts
            # on the scalar queue regressed: they delay the pT prefetch
            # behind 1.3MiB and skew the early pipeline).
            whT_sb = []
            hidT_sb = consts.tile([P, HC, BLOC], bf16)
            for j in range(AJ):
                wt = consts.tile([P, HC, P], bf16, name=f"whT{j}")
                nc.sync.dma_start(out=wt, in_=whTj[j, :, :, :])
                whT_sb.append(wt)
                if j == 0:
                    nc.sync.dma_start(out=hidT_sb, in_=hsT)
            waT_sb = consts.tile([P, AJ], bf16)
            nc.sync.dma_start(out=waT_sb, in_=waT)
            masks_all = consts.tile([P, BLOC, NT], bf16)
            nc.sync.dma_start(out=masks_all, in_=am)

            # pT rides the ACT HWDGE queue (SWDGE transfers measurably
            # degrade aggregate HBM bandwidth: ~320GB/s while pT went
            # through GPSIMD vs ~420GB/s af-only). Issued TWO cycles
            # ahead at the very start of a cycle; with exp demoted to
            # after the NEXT tanh batch and the transpose on GPSIMD,
            # nothing on the ACT queue can block these issues.
            def dma_patt(b):
                tiles = []
                for j in range(AJ):
                    pt = patt_pool.tile([P, N], pa_dt, tag="patt",
                                        name=f"patt{b}_{j}")
                    nc.scalar.dma_start(out=pt, in_=paT[b, j, :, :])
                    tiles.append(pt)
                return tiles

            def dma_af(b):
                tiles = []
                for st in range(NT // AF_SUP):
                    aft = af_pool.tile([P, AF_SUP, H], dt.float8e4,
                                       tag="af", name=f"af{b}_{st}")
                    nc.sync.dma_start(
                        out=aft,
                        in_=af_r[b][:, st * AF_SUP:(st + 1) * AF_SUP, :])
                    tiles.append(aft)
                return tiles

            pt_tiles = {0: dma_patt(0), 1: dma_patt(1)}
            af_tiles = {0: dma_af(0)}

            ones_col = consts.tile([P, 1], f32)
            nc.vector.memset(ones_col, 1.0)

            # w_hT[p, j, b] = sum_h W_h[j*128+p, h] * hidden[b, h]:
            # stationary W_h chunks put A on the OUTPUT partitions,
            # giving the transposed per-partition bias directly.
            whbias = consts.tile([P, AJ, BLOC], f32)
            for j in range(AJ):
                wh_ps = psc_pool.tile([P, BLOC], f32, tag="sc",
                                      name=f"whps{j}")
                for hc in range(HC):
                    nc.tensor.matmul(wh_ps, lhsT=whT_sb[j][:, hc, :],
                                     rhs=hidT_sb[:, hc, :],
                                     start=(hc == 0), stop=(hc == HC - 1))
                nc.scalar.copy(whbias[:, j, :], wh_ps)

            # ---------------- per-cycle phases ----------------
            def patt_compute(b):
                # fused add+tanh per a-chunk (data prefetched 2 cycles ago)
                alphas = []
                for j in range(AJ):
                    ab = alpha_pool.tile([P, N], bf16, tag="alpha",
                                         name=f"alpha{b}_{j}")
                    nc.scalar.activation(ab, pt_tiles[b][j], AF.Tanh,
                                         bias=whbias[:, j, b:b + 1])
                    alphas.append(ab)

                # scores on PE, issued j-OUTER so each tanh chunk feeds 4
                # matmuls immediately (the 4 PSUM banks' accumulation
                # groups interleave; each bank still sees start..stop in
                # order). Row nn = scores for n in [512nn, 512nn+512).
                scps = [psc_pool.tile([1, 512], f32, tag="sc",
                                      name=f"sc{b}_{nn}")
                        for nn in range(NN)]
                for j in range(AJ):
                    for nn in range(NN):
                        nc.tensor.matmul(
                            scps[nn], lhsT=waT_sb[:, j:j + 1],
                            rhs=alphas[j][:, nn * 512:(nn + 1) * 512],
                            start=(j == 0), stop=(j == AJ - 1))
                row4 = row_pool.tile([1, N], f32, tag="row", name=f"row{b}")
                for nn in range(NN):
                    nc.vector.tensor_copy(row4[:, nn * 512:(nn + 1) * 512],
                                          scps[nn])

                # tiny transpose DMA [1,2048] -> [128,16] (n = p*16+c) on
                # the GPSIMD queue so the ACT queue never waits on it
                scT = small.tile([P, NT], f32, tag="scT", name=f"scT{b}")
                nc.gpsimd.dma_start(out=scT, in_=row4)
                return scT

            def exp_phase(b, scT):
                # emitted AFTER tanh(b+1) in the ACT FIFO: the exp's only
                # dep (the transpose) is then a cycle old, so ACT never
                # blocks (an ACT stall here entrains tanh and collapses
                # the pipeline). Mask multiply + rowsum follow on DVE.
                # NOTE: the DVE stt cannot be emitted earlier than the
                # exp (tile deps follow program order; trying it read
                # uninitialized eraw -> nan).
                eraw = small.tile([P, NT], bf16, tag="eraw", name=f"eraw{b}")
                nc.scalar.activation(eraw, scT, AF.Exp)
                expt = small.tile([P, NT], bf16, tag="expt", name=f"expt{b}")
                rowsum = small.tile([P, 1], f32, tag="rowsum",
                                    name=f"rowsum{b}")
                nc.vector.scalar_tensor_tensor(
                    out=expt, in0=eraw, scalar=1.0, in1=masks_all[:, b, :],
                    op0=OP.mult, op1=OP.mult, accum_out=rowsum)
                return expt, rowsum

            def sum_phase(b, rowsum):
                # cycle b+1: softmax denominator; inv(b) is ready a full
                # cycle before the att drains need it
                sum_ps = psmisc.tile([1, 1], f32, tag="mm", name=f"sum{b}")
                nc.tensor.matmul(sum_ps, lhsT=rowsum, rhs=ones_col,
                                 start=True, stop=True)
                inv = small.tile([1, 1], f32, tag="inv", name=f"inv{b}")
                nc.vector.reciprocal(inv, sum_ps)
                return inv

            def att_mm(b, expt):
                # cycle b+2: every dependency (expt, af tiles) is >=1
                # cycle old, so the PE never waits here
                att_lo = psatt.tile([1, A], f32, tag="att", name=f"attlo{b}")
                att_hi = psatt.tile([1, A], f32, tag="att", name=f"atthi{b}")
                tiles = af_tiles.pop(b)  # loaded two cycles ago
                t = 0
                for st in range(NT // AF_SUP):
                    aft = tiles[st]
                    for c in range(AF_SUP):
                        lhs = expt[:, t:t + 1]
                        nc.tensor.matmul(att_lo, lhsT=lhs,
                                         rhs=aft[:, c, 0:A],
                                         start=(t == 0), stop=(t == NT - 1))
                        nc.tensor.matmul(att_hi, lhsT=lhs,
                                         rhs=aft[:, c, A:H],
                                         start=(t == 0), stop=(t == NT - 1))
                        t += 1
                return att_lo, att_hi

            def att_drain(b, att_lo, att_hi, inv):
                # drain on DVE with 1/sum folded in; emitted AFTER the
                # cycle's score copies + mask so the output path (never
                # urgent) sits last in the DVE FIFO and cannot delay the
                # score->exp chain
                att_row = arow_pool.tile([1, H], f32, tag="attrow",
                                         name=f"attrow{b}")
                nc.vector.tensor_scalar_mul(att_row[:, 0:A], att_lo, inv)
                nc.vector.tensor_scalar_mul(att_row[:, A:H], att_hi, inv)
                nc.gpsimd.dma_start(out=out[b:b + 1, :], in_=att_row)

            # ---------------- software pipeline (depth 2) ----------------
            # cycle k: sum(k-2) | att(k-2) | stream af(k)/pT(k+2) |
            # tanh(k)+scores(k) | exp(k-1).  The score->expt chain of
            # batch k overlaps the att matmuls of batch k-2 on the PE,
            # and every ACT/PE instruction's deps are >= 1 cycle old.
            scT_d, expt_d, rowsum_d, inv_d = {}, {}, {}, {}
            for k in range(BLOC + 2):
                if k + 2 < BLOC:
                    pt_tiles[k + 2] = dma_patt(k + 2)
                ps = None
                if k >= 2:
                    inv_d[k - 2] = sum_phase(k - 2, rowsum_d.pop(k - 2))
                    ps = att_mm(k - 2, expt_d.pop(k - 2))
                if k < BLOC:
                    if k >= 1:
                        af_tiles[k] = dma_af(k)
                    scT_d[k] = patt_compute(k)
                    pt_tiles.pop(k)
                if k >= 1 and k - 1 < BLOC:
                    expt_d[k - 1], rowsum_d[k - 1] = \
                        exp_phase(k - 1, scT_d.pop(k - 1))
                if ps is not None:
                    att_drain(k - 2, *ps, inv_d.pop(k - 2))

    nc.compile()
    return nc


def _get_nc():
    if "nc" not in _NC_CACHE:
        _NC_CACHE["nc"] = _build_nc()
    return _NC_CACHE["nc"]


def kernel(hidden_states, att_feats, p_att_feats, att_masks, W_h, W_alpha):
    import ml_dtypes
    from concourse.bass_utils import run_bass_kernel_spmd

    nc = _get_nc()
    pa_np = ml_dtypes.float8_e4m3fn if PA_FP8 else ml_dtypes.bfloat16
    hidden_states = np.ascontiguousarray(hidden_states, dtype=np.float32)
    att_feats = np.ascontiguousarray(att_feats, dtype=np.float32)
    p_att_feats = np.ascontiguousarray(p_att_feats, dtype=np.float32)
    att_masks = np.ascontiguousarray(att_masks, dtype=np.float32)
    W_h = np.ascontiguousarray(W_h, dtype=np.float32)
    W_alpha = np.asarray(W_alpha, dtype=np.float32)

    # W_hTj[j, p, hc, m] = W_h[j*128+m, hc*128+p]
    whTj = np.ascontiguousarray(
        W_h.reshape(AJ, P, HC, P).transpose(0, 3, 2, 1)
    ).astype(ml_dtypes.bfloat16)
    waT = np.ascontiguousarray(
        W_alpha.reshape(AJ, P).T).astype(ml_dtypes.bfloat16)

    in_maps = []
    for i in range(NCORES):
        s = slice(i * BLOC, (i + 1) * BLOC)
        # [H, BLOC] -> [P, HC, BLOC]
        hT = np.ascontiguousarray(
            hidden_states[s].T.reshape(HC, P, BLOC).transpose(1, 0, 2)
        ).astype(ml_dtypes.bfloat16)
        # exp(mask), [BLOC, N] -> [P, BLOC, NT] with n = p*16 + c
        amr = np.exp(att_masks[s].reshape(BLOC, P, NT).transpose(1, 0, 2)
                     ).astype(ml_dtypes.bfloat16)
        # [BLOC, N, A] -> [BLOC, AJ, P, N] with a = j*128 + p
        paT = p_att_feats[s].transpose(0, 2, 1).reshape(
            BLOC, AJ, P, N).astype(pa_np)
        in_maps.append({
            "p_att_T": paT,
            "att_feats": att_feats[s].astype(pa_np),
            "hidden_T": hT,
            "att_masks": amr,
            "W_hTj": whTj,
            "W_alphaT": waT,
        })

    global _LAST_IN_MAPS
    _LAST_IN_MAPS = in_maps
    res = run_bass_kernel_spmd(nc, in_maps, core_ids=list(range(NCORES)))
    return np.concatenate(
        [res.results[i]["att_out"] for i in range(NCORES)], axis=0
    ).astype(np.float32)


_LAST_IN_MAPS = None


# revision 40
# speedup vs baseline: 1.2040x; 1.0109x over previous
"""Bass/Tile TRN2 kernel for BasicAttention (low-precision streams,
PE-based scores, depth-2 prefetched software pipeline).

att = softmax(tanh(hidden @ W_h.T + p_att_feats) @ W_alpha + mask) @ att_feats

Shapes: B=64, N=2048, H=1024, A=512. Data-parallel over batch across 8
NeuronCores (8 batches per core); weights replicated; no collectives.
Measured ~125-130us/core (baseline all-f32 was 259-310us); rel err 1.83e-2.

Design:
  * host casts the streams: att_feats AND p_att_feats -> fp8e4m3
    (device rel err ~1.83e-2 vs the 2e-2 budget; deterministic and
    matching the numpy ml_dtypes simulation within 0.2%). The att
    matmuls run mixed bf16(lhsT) x fp8(rhs). HBM: 102MB -> ~26MB/core;
    the PE (~95us) becomes the binding budget instead of DMA.
  * p_att host-transposed to [b, j, a=128p, n]: A on partitions, so the
    w_h add is a per-partition scalar and add+tanh fuse into ONE ACT op
    (bias=w_hT column). No DVE elementwise add anywhere.
  * scores = alpha.T @ W_alpha on PE (lhsT=W_alphaT column, rhs=alpha
    chunk; j-outer issue so each tanh chunk feeds 4 matmuls at once;
    4 a-chunks accumulate into four [1,512] PSUM bank rows).
  * score rows drain via DVE copies to a [1,2048] row; one tiny
    GPSIMD-queue DMA transposes it to the region-partition layout
    [128,16] (n = p*16+c) needed as att-matmul lhsT.
  * masking is MULTIPLICATIVE after exp (host sends exp(mask); exact
    since exp(s+m)=exp(s)exp(m)): the ACT exp depends only on the
    transpose, and the DVE stt does expt=eraw*expmask with the rowsum
    accumulated in the same op. Softmax denominator via PE ones-matmul,
    reciprocal on DVE, folded into the DVE PSUM drains; SWDGE output.
  * depth-2 pipeline, both streams prefetched on separate HWDGE queues
    (af on Sync, pT on the ACT queue, issued 2 cycles ahead at cycle
    start). Per cycle k: PE [sum(k-2), att(k-2) x32, scores(k) x16],
    ACT [pT(k+2) issues, tanh(k) x4, exp(k-1)] -- every instruction's
    deps are >= 1 cycle old, so no engine FIFO ever blocks on the
    score chain (any such wait entrains tanh and collapses the
    pipeline; this ordering was worth ~40us).

Engine budget/core per ~12-14us cycle: DMA 3.2MiB (~8us at 420GB/s),
PE ~11us (now the binding budget), ACT ~11us, DVE ~5us.

Layouts: region n = p*16 + c (att_feats/masks/scores); a = j*128 + p
(p_att/W_h).
"""

import numpy as np

B, N, H, A = 64, 2048, 1024, 512
NCORES = 8
BLOC = B // NCORES  # batches per core

P = 128
NT = N // P            # 16 n-columns per partition (att layout)
AJ = A // P            # 4 a-chunks (p_att layout)
NN = N // 512          # 4 score chunks of 512
HC = H // P            # 8 contraction chunks for the w_h matmul
AF_SUP = 4             # columns per att_feats supertile (4 DMAs per batch)

# stream dtype for p_att (and att_feats): fp8e4m3. Device rel err
# 1.832e-2 vs the 2e-2 gate, bit-stable and equal to the numpy
# ml_dtypes simulation; flip to bf16 (rel 2.2e-3, ~+25us) if the
# inputs ever change character.
PA_FP8 = True

_NC_CACHE = {}


def _build_nc():
    import concourse.bass as bass
    import concourse.mybir as mybir
    import concourse.tile as tile
    from concourse import bacc

    dt = mybir.dt
    f32, bf16 = dt.float32, dt.bfloat16
    pa_dt = dt.float8e4 if PA_FP8 else bf16
    AF = mybir.ActivationFunctionType
    OP = mybir.AluOpType

    nc = bacc.Bacc("TRN2", target_bir_lowering=False, debug=False,
                   num_devices=NCORES)

    paT = nc.dram_tensor("p_att_T", [BLOC, AJ, P, N], pa_dt,
                         kind="ExternalInput").ap()
    af = nc.dram_tensor("att_feats", [BLOC, N, H], dt.float8e4,
                        kind="ExternalInput").ap()
    whTj = nc.dram_tensor("W_hTj", [AJ, P, HC, P], bf16,
                          kind="ExternalInput").ap()
    hsT = nc.dram_tensor("hidden_T", [P, HC, BLOC], bf16,
                         kind="ExternalInput").ap()
    # exp(att_masks): multiplicative masking (exp(s+m) = exp(s)*exp(m),
    # exact; zeros -> 1.0) applied AFTER exp on DVE, so the ACT exp
    # depends only on the transpose DMA -- not on any late DVE stage
    am = nc.dram_tensor("exp_masks", [P, BLOC, NT], bf16,
                        kind="ExternalInput").ap()
    waT = nc.dram_tensor("W_alphaT", [P, AJ], bf16, kind="ExternalInput").ap()
    out = nc.dram_tensor("att_out", [BLOC, H], f32, kind="ExternalOutput").ap()

    with tile.TileContext(nc) as tc:
        with (
            tc.tile_pool(name="consts", bufs=1) as consts,
            tc.tile_pool(name="patt", bufs=12) as patt_pool,
            tc.tile_pool(name="alpha", bufs=5) as alpha_pool,
            tc.tile_pool(name="afp", bufs=14) as af_pool,
            tc.tile_pool(name="small", bufs=3) as small,
            tc.tile_pool(name="rowp", bufs=2) as row_pool,
            tc.tile_pool(name="arow", bufs=2) as arow_pool,
            tc.tile_pool(name="psc", bufs=4, space="PSUM") as psc_pool,
            tc.tile_pool(name="psatt", bufs=3, space="PSUM") as psatt,
            tc.tile_pool(name="psmisc", bufs=1, space="PSUM") as psmisc,
        ):
            af_r = [af[b, :, :].rearrange("(p c) h -> p c h", c=NT)
                    for b in range(BLOC)]

            # ---------------- prologue ----------------
            # Sync queue: consts then the first att_feats batch (consts
            # on the scalar queue regressed: they delay the pT prefetch
            # behind 1.3MiB and skew the early pipeline).
            whT_sb = []
            hidT_sb = consts.tile([P, HC, BLOC], bf16)
            for j in range(AJ):
                wt = consts.tile([P, HC, P], bf16, name=f"whT{j}")
                nc.sync.dma_start(out=wt, in_=whTj[j, :, :, :])
                whT_sb.append(wt)
                if j == 0:
                    nc.sync.dma_start(out=hidT_sb, in_=hsT)
            waT_sb = consts.tile([P, AJ], bf16)
            nc.sync.dma_start(out=waT_sb, in_=waT)
            masks_all = consts.tile([P, BLOC, NT], bf16)
            nc.sync.dma_start(out=masks_all, in_=am)

            # pT rides the ACT HWDGE queue (SWDGE transfers measurably
            # degrade aggregate HBM bandwidth: ~320GB/s while pT went
            # through GPSIMD vs ~420GB/s af-only). Issued TWO cycles
            # ahead at the very start of a cycle; with exp demoted to
            # after the NEXT tanh batch and the transpose on GPSIMD,
            # nothing on the ACT queue can block these issues.
            def dma_patt(b):
                tiles = []
                for j in range(AJ):
                    pt = patt_pool.tile([P, N], pa_dt, tag="patt",
                                        name=f"patt{b}_{j}")
                    nc.scalar.dma_start(out=pt, in_=paT[b, j, :, :])
                    tiles.append(pt)
                return tiles

            def dma_af(b):
                tiles = []
                for st in range(NT // AF_SUP):
                    aft = af_pool.tile([P, AF_SUP, H], dt.float8e4,
                                       tag="af", name=f"af{b}_{st}")
                    nc.sync.dma_start(
                        out=aft,
                        in_=af_r[b][:, st * AF_SUP:(st + 1) * AF_SUP, :])
                    tiles.append(aft)
                return tiles

            pt_tiles = {0: dma_patt(0), 1: dma_patt(1)}
            af_tiles = {0: dma_af(0)}

            ones_col = consts.tile([P, 1], f32)
            nc.vector.memset(ones_col, 1.0)

            # w_hT[p, j, b] = sum_h W_h[j*128+p, h] * hidden[b, h]:
            # stationary W_h chunks put A on the OUTPUT partitions,
            # giving the transposed per-partition bias directly.
            whbias = consts.tile([P, AJ, BLOC], f32)
            for j in range(AJ):
                wh_ps = psc_pool.tile([P, BLOC], f32, tag="sc",
                                      name=f"whps{j}")
                for hc in range(HC):
                    nc.tensor.matmul(wh_ps, lhsT=whT_sb[j][:, hc, :],
                                     rhs=hidT_sb[:, hc, :],
                                     start=(hc == 0), stop=(hc == HC - 1))
                nc.vector.tensor_copy(whbias[:, j, :], wh_ps)

            # ---------------- per-cycle phases ----------------
            def patt_compute(b):
                # fused add+tanh per a-chunk (data prefetched 2 cycles ago)
                alphas = []
                for j in range(AJ):
                    ab = alpha_pool.tile([P, N], bf16, tag="alpha",
                                         name=f"alpha{b}_{j}")
                    nc.scalar.activation(ab, pt_tiles[b][j], AF.Tanh,
                                         bias=whbias[:, j, b:b + 1])
                    alphas.append(ab)

                # scores on PE, issued j-OUTER so each tanh chunk feeds 4
                # matmuls immediately (the 4 PSUM banks' accumulation
                # groups interleave; each bank still sees start..stop in
                # order). Row nn = scores for n in [512nn, 512nn+512).
                scps = [psc_pool.tile([1, 512], f32, tag="sc",
                                      name=f"sc{b}_{nn}")
                        for nn in range(NN)]
                for j in range(AJ):
                    for nn in range(NN):
                        nc.tensor.matmul(
                            scps[nn], lhsT=waT_sb[:, j:j + 1],
                            rhs=alphas[j][:, nn * 512:(nn + 1) * 512],
                            start=(j == 0), stop=(j == AJ - 1))
                row4 = row_pool.tile([1, N], f32, tag="row", name=f"row{b}")
                for nn in range(NN):
                    nc.vector.tensor_copy(row4[:, nn * 512:(nn + 1) * 512],
                                          scps[nn])

                # tiny transpose DMA [1,2048] -> [128,16] (n = p*16+c) on
                # the GPSIMD queue so the ACT queue never waits on it
                scT = small.tile([P, NT], f32, tag="scT", name=f"scT{b}")
                nc.gpsimd.dma_start(out=scT, in_=row4)
                return scT

            def exp_phase(b, scT):
                # emitted AFTER tanh(b+1) in the ACT FIFO: the exp's only
                # dep (the transpose) is then a cycle old, so ACT never
                # blocks (an ACT stall here entrains tanh and collapses
                # the pipeline). Mask multiply + rowsum follow on DVE.
                # NOTE: the DVE stt cannot be emitted earlier than the
                # exp (tile deps follow program order; trying it read
                # uninitialized eraw -> nan).
                eraw = small.tile([P, NT], bf16, tag="eraw", name=f"eraw{b}")
                nc.scalar.activation(eraw, scT, AF.Exp)
                expt = small.tile([P, NT], bf16, tag="expt", name=f"expt{b}")
                rowsum = small.tile([P, 1], f32, tag="rowsum",
                                    name=f"rowsum{b}")
                nc.vector.scalar_tensor_tensor(
                    out=expt, in0=eraw, scalar=1.0, in1=masks_all[:, b, :],
                    op0=OP.mult, op1=OP.mult, accum_out=rowsum)
                return expt, rowsum

            def sum_phase(b, rowsum):
                # cycle b+1: softmax denominator; inv(b) is ready a full
                # cycle before the att drains need it
                sum_ps = psmisc.tile([1, 1], f32, tag="mm", name=f"sum{b}")
                nc.tensor.matmul(sum_ps, lhsT=rowsum, rhs=ones_col,
                                 start=True, stop=True)
                inv = small.tile([1, 1], f32, tag="inv", name=f"inv{b}")
                nc.vector.reciprocal(inv, sum_ps)
                return inv

            def att_mm(b, expt):
                # cycle b+2: every dependency (expt, af tiles) is >=1
                # cycle old, so the PE never waits here
                att_lo = psatt.tile([1, A], f32, tag="att", name=f"attlo{b}")
                att_hi = psatt.tile([1, A], f32, tag="att", name=f"atthi{b}")
                tiles = af_tiles.pop(b)  # loaded two cycles ago
                t = 0
                for st in range(NT // AF_SUP):
                    aft = tiles[st]
                    for c in range(AF_SUP):
                        lhs = expt[:, t:t + 1]
                        nc.tensor.matmul(att_lo, lhsT=lhs,
                                         rhs=aft[:, c, 0:A],
                                         start=(t == 0), stop=(t == NT - 1))
                        nc.tensor.matmul(att_hi, lhsT=lhs,
                                         rhs=aft[:, c, A:H],
                                         start=(t == 0), stop=(t == NT - 1))
                        t += 1
                return att_lo, att_hi

            def att_drain(b, att_lo, att_hi, inv):
                # drain on DVE with 1/sum folded in; emitted AFTER the
                # cycle's score copies + mask so the output path (never
                # urgent) sits last in the DVE FIFO and cannot delay the
                # score->exp chain
                att_row = arow_pool.tile([1, H], f32, tag="attrow",
                                         name=f"attrow{b}")
                nc.vector.tensor_scalar_mul(att_row[:, 0:A], att_lo, inv)
                nc.vector.tensor_scalar_mul(att_row[:, A:H], att_hi, inv)
                nc.gpsimd.dma_start(out=out[b:b + 1, :], in_=att_row)

            # ---------------- software pipeline (depth 2) ----------------
            # cycle k: sum(k-2) | att(k-2) | stream af(k)/pT(k+2) |
            # tanh(k)+scores(k) | exp(k-1).  The score->expt chain of
            # batch k overlaps the att matmuls of batch k-2 on the PE,
            # and every ACT/PE instruction's deps are >= 1 cycle old.
            scT_d, expt_d, rowsum_d, inv_d = {}, {}, {}, {}
            for k in range(BLOC + 2):
                if k + 2 < BLOC:
                    pt_tiles[k + 2] = dma_patt(k + 2)
                ps = None
                if k >= 2:
                    inv_d[k - 2] = sum_phase(k - 2, rowsum_d.pop(k - 2))
                    ps = att_mm(k - 2, expt_d.pop(k - 2))
                if k < BLOC:
                    if k >= 1:
                        af_tiles[k] = dma_af(k)
                    scT_d[k] = patt_compute(k)
                    pt_tiles.pop(k)
                if k >= 1 and k - 1 < BLOC:
                    expt_d[k - 1], rowsum_d[k - 1] = \
                        exp_phase(k - 1, scT_d.pop(k - 1))
                if ps is not None:
                    att_drain(k - 2, *ps, inv_d.pop(k - 2))

    nc.compile()
    return nc


def _get_nc():
    if "nc" not in _NC_CACHE:
        _NC_CACHE["nc"] = _build_nc()
    return _NC_CACHE["nc"]


def kernel(hidden_states, att_feats, p_att_feats, att_masks, W_h, W_alpha):
    import ml_dtypes
    from concourse.bass_utils import run_bass_kernel_spmd

    nc = _get_nc()
    pa_np = ml_dtypes.float8_e4m3fn if PA_FP8 else ml_dtypes.bfloat16
    hidden_states = np.ascontiguousarray(hidden_states, dtype=np.float32)
    att_feats = np.ascontiguousarray(att_feats, dtype=np.float32)
    p_att_feats = np.ascontiguousarray(p_att_feats, dtype=np.float32)
    att_masks = np.ascontiguousarray(att_masks, dtype=np.float32)
    W_h = np.ascontiguousarray(W_h, dtype=np.float32)
    W_alpha = np.asarray(W_alpha, dtype=np.float32)

    # W_hTj[j, p, hc, m] = W_h[j*128+m, hc*128+p]
    whTj = np.ascontiguousarray(
        W_h.reshape(AJ, P, HC, P).transpose(0, 3, 2, 1)
    ).astype(ml_dtypes.bfloat16)
    waT = np.ascontiguousarray(
        W_alpha.reshape(AJ, P).T).astype(ml_dtypes.bfloat16)

    in_maps = []
    for i in range(NCORES):
        s = slice(i * BLOC, (i + 1) * BLOC)
        # [H, BLOC] -> [P, HC, BLOC]
        hT = np.ascontiguousarray(
            hidden_states[s].T.reshape(HC, P, BLOC).transpose(1, 0, 2)
        ).astype(ml_dtypes.bfloat16)
        # exp(mask), [BLOC, N] -> [P, BLOC, NT] with n = p*16 + c
        amr = np.exp(att_masks[s].reshape(BLOC, P, NT).transpose(1, 0, 2)
                     ).astype(ml_dtypes.bfloat16)
        # [BLOC, N, A] -> [BLOC, AJ, P, N] with a = j*128 + p
        paT = p_att_feats[s].transpose(0, 2, 1).reshape(
            BLOC, AJ, P, N).astype(pa_np)
        in_maps.append({
            "p_att_T": paT,
            "att_feats": att_feats[s].astype(pa_np),
            "hidden_T": hT,
            "att_masks": amr,
            "W_hTj": whTj,
            "W_alphaT": waT,
        })

    global _LAST_IN_MAPS
    _LAST_IN_MAPS = in_maps
    res = run_bass_kernel_spmd(nc, in_maps, core_ids=list(range(NCORES)))
    return np.concatenate(
        [res.results[i]["att_out"] for i in range(NCORES)], axis=0
    ).astype(np.float32)


_LAST_IN_MAPS = None


# revision 41
# speedup vs baseline: 1.2497x; 1.0379x over previous
"""Bass/Tile TRN2 kernel for BasicAttention (low-precision streams,
PE-based scores, depth-2 prefetched software pipeline).

att = softmax(tanh(hidden @ W_h.T + p_att_feats) @ W_alpha + mask) @ att_feats

Shapes: B=64, N=2048, H=1024, A=512. Data-parallel over batch across 8
NeuronCores (8 batches per core); weights replicated; no collectives.
Measured ~125-130us/core (baseline all-f32 was 259-310us); rel err 1.83e-2.

Design:
  * host casts the streams: att_feats AND p_att_feats -> fp8e4m3
    (device rel err ~1.83e-2 vs the 2e-2 budget; deterministic and
    matching the numpy ml_dtypes simulation within 0.2%). The att
    matmuls run mixed bf16(lhsT) x fp8(rhs). HBM: 102MB -> ~26MB/core;
    the PE (~95us) becomes the binding budget instead of DMA.
  * p_att host-transposed to [b, j, a=128p, n]: A on partitions, so the
    w_h add is a per-partition scalar and add+tanh fuse into ONE ACT op
    (bias=w_hT column). No DVE elementwise add anywhere.
  * scores = alpha.T @ W_alpha on PE (lhsT=W_alphaT column, rhs=alpha
    chunk; j-outer issue so each tanh chunk feeds 4 matmuls at once;
    4 a-chunks accumulate into four [1,512] PSUM bank rows).
  * score rows drain via DVE copies to a [1,2048] row; one tiny
    GPSIMD-queue DMA transposes it to the region-partition layout
    [128,16] (n = p*16+c) needed as att-matmul lhsT.
  * masking is MULTIPLICATIVE after exp (host sends exp(mask); exact
    since exp(s+m)=exp(s)exp(m)): the ACT exp depends only on the
    transpose, and the DVE stt does expt=eraw*expmask with the rowsum
    accumulated in the same op. Softmax denominator via PE ones-matmul,
    reciprocal on DVE, folded into the DVE PSUM drains; SWDGE output.
  * depth-2 pipeline, both streams prefetched on separate HWDGE queues
    (af on Sync, pT on the ACT queue, issued 2 cycles ahead at cycle
    start). Per cycle k: PE [sum(k-2), att(k-2) x32, scores(k) x16],
    ACT [pT(k+2) issues, tanh(k) x4, exp(k-1)] -- every instruction's
    deps are >= 1 cycle old, so no engine FIFO ever blocks on the
    score chain (any such wait entrains tanh and collapses the
    pipeline; this ordering was worth ~40us).

Engine budget/core per ~12-14us cycle: DMA 3.2MiB (~8us at 420GB/s),
PE ~11us (now the binding budget), ACT ~11us, DVE ~5us.

Layouts: region n = p*16 + c (att_feats/masks/scores); a = j*128 + p
(p_att/W_h).
"""

import numpy as np

B, N, H, A = 64, 2048, 1024, 512
NCORES = 8
BLOC = B // NCORES  # batches per core

P = 128
NT = N // P            # 16 n-columns per partition (att layout)
AJ = A // P            # 4 a-chunks (p_att layout)
NN = N // 512          # 4 score chunks of 512
HC = H // P            # 8 contraction chunks for the w_h matmul
AF_SUP = 4             # columns per att_feats supertile (4 DMAs per batch)

# stream dtype for p_att (and att_feats): fp8e4m3. Device rel err
# 1.832e-2 vs the 2e-2 gate, bit-stable and equal to the numpy
# ml_dtypes simulation; flip to bf16 (rel 2.2e-3, ~+25us) if the
# inputs ever change character.
PA_FP8 = True

_NC_CACHE = {}


def _build_nc():
    import concourse.bass as bass
    import concourse.mybir as mybir
    import concourse.tile as tile
    from concourse import bacc

    dt = mybir.dt
    f32, bf16 = dt.float32, dt.bfloat16
    pa_dt = dt.float8e4 if PA_FP8 else bf16
    AF = mybir.ActivationFunctionType
    OP = mybir.AluOpType

    nc = bacc.Bacc("TRN2", target_bir_lowering=False, debug=False,
                   num_devices=NCORES)

    paT = nc.dram_tensor("p_att_T", [BLOC, AJ, P, N], pa_dt,
                         kind="ExternalInput").ap()
    af = nc.dram_tensor("att_feats", [BLOC, N, H], dt.float8e4,
                        kind="ExternalInput").ap()
    whTj = nc.dram_tensor("W_hTj", [AJ, P, HC, P], bf16,
                          kind="ExternalInput").ap()
    hsT = nc.dram_tensor("hidden_T", [P, HC, BLOC], bf16,
                         kind="ExternalInput").ap()
    # exp(att_masks): multiplicative masking (exp(s+m) = exp(s)*exp(m),
    # exact; zeros -> 1.0) applied AFTER exp on DVE, so the ACT exp
    # depends only on the transpose DMA -- not on any late DVE stage
    am = nc.dram_tensor("exp_masks", [P, BLOC, NT], bf16,
                        kind="ExternalInput").ap()
    waT = nc.dram_tensor("W_alphaT", [P, AJ], bf16, kind="ExternalInput").ap()
    out = nc.dram_tensor("att_out", [BLOC, H], f32, kind="ExternalOutput").ap()

    with tile.TileContext(nc) as tc:
        with (
            tc.tile_pool(name="consts", bufs=1) as consts,
            tc.tile_pool(name="patt", bufs=12) as patt_pool,
            tc.tile_pool(name="alpha", bufs=5) as alpha_pool,
            tc.tile_pool(name="afp", bufs=14) as af_pool,
            tc.tile_pool(name="small", bufs=3) as small,
            tc.tile_pool(name="rowp", bufs=2) as row_pool,
            tc.tile_pool(name="arow", bufs=2) as arow_pool,
            tc.tile_pool(name="psc", bufs=4, space="PSUM") as psc_pool,
            tc.tile_pool(name="psatt", bufs=3, space="PSUM") as psatt,
            tc.tile_pool(name="psmisc", bufs=1, space="PSUM") as psmisc,
        ):
            af_r = [af[b, :, :].rearrange("(p c) h -> p c h", c=NT)
                    for b in range(BLOC)]

            # ---------------- prologue ----------------
            # Sync queue: consts then the first att_feats batch (consts
            # on the scalar queue regressed: they delay the pT prefetch
            # behind 1.3MiB and skew the early pipeline).
            whT_sb = []
            hidT_sb = consts.tile([P, HC, BLOC], bf16)
            for j in range(AJ):
                wt = consts.tile([P, HC, P], bf16, name=f"whT{j}")
                nc.sync.dma_start(out=wt, in_=whTj[j, :, :, :])
                whT_sb.append(wt)
                if j == 0:
                    nc.sync.dma_start(out=hidT_sb, in_=hsT)
            waT_sb = consts.tile([P, AJ], bf16)
            nc.sync.dma_start(out=waT_sb, in_=waT)
            masks_all = consts.tile([P, BLOC, NT], bf16)
            nc.sync.dma_start(out=masks_all, in_=am)

            # pT rides the ACT HWDGE queue (SWDGE transfers measurably
            # degrade aggregate HBM bandwidth: ~320GB/s while pT went
            # through GPSIMD vs ~420GB/s af-only). Issued TWO cycles
            # ahead at the very start of a cycle; with exp demoted to
            # after the NEXT tanh batch and the transpose on GPSIMD,
            # nothing on the ACT queue can block these issues.
            def dma_patt(b):
                tiles = []
                for j in range(AJ):
                    pt = patt_pool.tile([P, N], pa_dt, tag="patt",
                                        name=f"patt{b}_{j}")
                    nc.scalar.dma_start(out=pt, in_=paT[b, j, :, :])
                    tiles.append(pt)
                return tiles

            def dma_af(b):
                tiles = []
                for st in range(NT // AF_SUP):
                    aft = af_pool.tile([P, AF_SUP, H], dt.float8e4,
                                       tag="af", name=f"af{b}_{st}")
                    nc.sync.dma_start(
                        out=aft,
                        in_=af_r[b][:, st * AF_SUP:(st + 1) * AF_SUP, :])
                    tiles.append(aft)
                return tiles

            pt_tiles = {0: dma_patt(0), 1: dma_patt(1)}
            af_tiles = {0: dma_af(0)}

            ones_col = consts.tile([P, 1], f32)
            nc.vector.memset(ones_col, 1.0)

            # w_hT[p, j, b] = sum_h W_h[j*128+p, h] * hidden[b, h]:
            # stationary W_h chunks put A on the OUTPUT partitions,
            # giving the transposed per-partition bias directly.
            whbias = consts.tile([P, AJ, BLOC], f32)
            for j in range(AJ):
                wh_ps = psc_pool.tile([P, BLOC], f32, tag="sc",
                                      name=f"whps{j}")
                for hc in range(HC):
                    nc.tensor.matmul(wh_ps, lhsT=whT_sb[j][:, hc, :],
                                     rhs=hidT_sb[:, hc, :],
                                     start=(hc == 0), stop=(hc == HC - 1))
                nc.vector.tensor_copy(whbias[:, j, :], wh_ps)

            # ---------------- per-cycle phases ----------------
            def patt_compute(b):
                # fused add+tanh per a-chunk (data prefetched 2 cycles ago)
                alphas = []
                for j in range(AJ):
                    ab = alpha_pool.tile([P, N], bf16, tag="alpha",
                                         name=f"alpha{b}_{j}")
                    nc.scalar.activation(ab, pt_tiles[b][j], AF.Tanh,
                                         bias=whbias[:, j, b:b + 1])
                    alphas.append(ab)

                # scores on PE, issued j-OUTER so each tanh chunk feeds 4
                # matmuls immediately (the 4 PSUM banks' accumulation
                # groups interleave; each bank still sees start..stop in
                # order). Row nn = scores for n in [512nn, 512nn+512).
                scps = [psc_pool.tile([1, 512], f32, tag="sc",
                                      name=f"sc{b}_{nn}")
                        for nn in range(NN)]
                for j in range(AJ):
                    for nn in range(NN):
                        nc.tensor.matmul(
                            scps[nn], lhsT=waT_sb[:, j:j + 1],
                            rhs=alphas[j][:, nn * 512:(nn + 1) * 512],
                            start=(j == 0), stop=(j == AJ - 1))
                row4 = row_pool.tile([1, N], f32, tag="row", name=f"row{b}")
                for nn in range(NN):
                    nc.vector.tensor_copy(row4[:, nn * 512:(nn + 1) * 512],
                                          scps[nn])

                # tiny transpose DMA [1,2048] -> [128,16] (n = p*16+c) on
                # the GPSIMD queue so the ACT queue never waits on it
                scT = small.tile([P, NT], f32, tag="scT", name=f"scT{b}")
                nc.gpsimd.dma_start(out=scT, in_=row4)
                return scT

            def exp_phase(b, scT):
                # emitted AFTER tanh(b+1) in the ACT FIFO: the exp's only
                # dep (the transpose) is then a cycle old, so ACT never
                # blocks (an ACT stall here entrains tanh and collapses
                # the pipeline). Mask multiply + rowsum follow on DVE.
                # NOTE: the DVE stt cannot be emitted earlier than the
                # exp (tile deps follow program order; trying it read
                # uninitialized eraw -> nan).
                eraw = small.tile([P, NT], bf16, tag="eraw", name=f"eraw{b}")
                nc.scalar.activation(eraw, scT, AF.Exp)
                expt = small.tile([P, NT], bf16, tag="expt", name=f"expt{b}")
                rowsum = small.tile([P, 1], f32, tag="rowsum",
                                    name=f"rowsum{b}")
                nc.vector.scalar_tensor_tensor(
                    out=expt, in0=eraw, scalar=1.0, in1=masks_all[:, b, :],
                    op0=OP.mult, op1=OP.mult, accum_out=rowsum)
                return expt, rowsum

            def sum_phase(b, rowsum):
                # cycle b+1: softmax denominator; inv(b) is ready a full
                # cycle before the att drains need it
                sum_ps = psmisc.tile([1, 1], f32, tag="mm", name=f"sum{b}")
                nc.tensor.matmul(sum_ps, lhsT=rowsum, rhs=ones_col,
                                 start=True, stop=True)
                inv = small.tile([1, 1], f32, tag="inv", name=f"inv{b}")
                nc.vector.reciprocal(inv, sum_ps)
                return inv

            def att_mm(b, expt):
                # cycle b+2: every dependency (expt, af tiles) is >=1
                # cycle old, so the PE never waits here
                att_lo = psatt.tile([1, A], f32, tag="att", name=f"attlo{b}")
                att_hi = psatt.tile([1, A], f32, tag="att", name=f"atthi{b}")
                tiles = af_tiles.pop(b)  # loaded two cycles ago
                t = 0
                for st in range(NT // AF_SUP):
                    aft = tiles[st]
                    for c in range(AF_SUP):
                        lhs = expt[:, t:t + 1]
                        nc.tensor.matmul(att_lo, lhsT=lhs,
                                         rhs=aft[:, c, 0:A],
                                         start=(t == 0), stop=(t == NT - 1))
                        nc.tensor.matmul(att_hi, lhsT=lhs,
                                         rhs=aft[:, c, A:H],
                                         start=(t == 0), stop=(t == NT - 1))
                        t += 1
                return att_lo, att_hi

            def att_drain(b, att_lo, att_hi, inv):
                # drain on DVE with 1/sum folded in; emitted AFTER the
                # cycle's score copies + mask so the output path (never
                # urgent) sits last in the DVE FIFO and cannot delay the
                # score->exp chain
                att_row = arow_pool.tile([1, H], f32, tag="attrow",
                                         name=f"attrow{b}")
                nc.vector.tensor_scalar_mul(att_row[:, 0:A], att_lo, inv)
                nc.vector.tensor_scalar_mul(att_row[:, A:H], att_hi, inv)
                nc.gpsimd.dma_start(out=out[b:b + 1, :], in_=att_row)

            # ---------------- software pipeline (depth 2) ----------------
            # cycle k: sum(k-2) | att(k-2) | stream af(k)/pT(k+2) |
            # tanh(k)+scores(k) | exp(k-1).  The score->expt chain of
            # batch k overlaps the att matmuls of batch k-2 on the PE,
            # and every ACT/PE instruction's deps are >= 1 cycle old.
            scT_d, expt_d, rowsum_d, inv_d = {}, {}, {}, {}
            for k in range(BLOC + 2):
                ps = None
                if k >= 2:
                    inv_d[k - 2] = sum_phase(k - 2, rowsum_d.pop(k - 2))
                    ps = att_mm(k - 2, expt_d.pop(k - 2))
                if k < BLOC:
                    if k >= 1:
                        af_tiles[k] = dma_af(k)
                    scT_d[k] = patt_compute(k)
                    pt_tiles.pop(k)
                if k >= 1 and k - 1 < BLOC:
                    expt_d[k - 1], rowsum_d[k - 1] = \
                        exp_phase(k - 1, scT_d.pop(k - 1))
                # pT(k+2) issues AFTER the cycle's tanh/exp on the ACT
                # FIFO: still 2 cycles ahead of use, but they can no
                # longer delay tanh (the fill showed 12 queued pT issues
                # holding tanh(0) until 20us while its deps were ready
                # at 12us)
                if k + 2 < BLOC:
                    pt_tiles[k + 2] = dma_patt(k + 2)
                if ps is not None:
                    att_drain(k - 2, *ps, inv_d.pop(k - 2))

    nc.compile()
    return nc


def _get_nc():
    if "nc" not in _NC_CACHE:
        _NC_CACHE["nc"] = _build_nc()
    return _NC_CACHE["nc"]


def kernel(hidden_states, att_feats, p_att_feats, att_masks, W_h, W_alpha):
    import ml_dtypes
    from concourse.bass_utils import run_bass_kernel_spmd

    nc = _get_nc()
    pa_np = ml_dtypes.float8_e4m3fn if PA_FP8 else ml_dtypes.bfloat16
    hidden_states = np.ascontiguousarray(hidden_states, dtype=np.float32)
    att_feats = np.ascontiguousarray(att_feats, dtype=np.float32)
    p_att_feats = np.ascontiguousarray(p_att_feats, dtype=np.float32)
    att_masks = np.ascontiguousarray(att_masks, dtype=np.float32)
    W_h = np.ascontiguousarray(W_h, dtype=np.float32)
    W_alpha = np.asarray(W_alpha, dtype=np.float32)

    # W_hTj[j, p, hc, m] = W_h[j*128+m, hc*128+p]
    whTj = np.ascontiguousarray(
        W_h.reshape(AJ, P, HC, P).transpose(0, 3, 2, 1)
    ).astype(ml_dtypes.bfloat16)
    waT = np.ascontiguousarray(
        W_alpha.reshape(AJ, P).T).astype(ml_dtypes.bfloat16)

    in_maps = []
    for i in range(NCORES):
        s = slice(i * BLOC, (i + 1) * BLOC)
        # [H, BLOC] -> [P, HC, BLOC]
        hT = np.ascontiguousarray(
            hidden_states[s].T.reshape(HC, P, BLOC).transpose(1, 0, 2)
        ).astype(ml_dtypes.bfloat16)
        # exp(mask), [BLOC, N] -> [P, BLOC, NT] with n = p*16 + c
        amr = np.exp(att_masks[s].reshape(BLOC, P, NT).transpose(1, 0, 2)
                     ).astype(ml_dtypes.bfloat16)
        # [BLOC, N, A] -> [BLOC, AJ, P, N] with a = j*128 + p
        paT = p_att_feats[s].transpose(0, 2, 1).reshape(
            BLOC, AJ, P, N).astype(pa_np)
        in_maps.append({
            "p_att_T": paT,
            "att_feats": att_feats[s].astype(pa_np),
            "hidden_T": hT,
            "att_masks": amr,
            "W_hTj": whTj,
            "W_alphaT": waT,
        })

    global _LAST_IN_MAPS
    _LAST_IN_MAPS = in_maps
    res = run_bass_kernel_spmd(nc, in_maps, core_ids=list(range(NCORES)))
    return np.concatenate(
        [res.results[i]["att_out"] for i in range(NCORES)], axis=0
    ).astype(np.float32)


_LAST_IN_MAPS = None
